# revision 26
# baseline (speedup 1.0000x reference)
"""Trainium2 Bass kernel: single-head self-attention.

Reference computation (fp32):
    q = x @ Wq.T ; k = x @ Wk.T ; v = x @ Wv.T        (x: [4, 2048, 1024])
    out = softmax((q @ k.T) / 32) @ v                 ([4, 2048, 1024])

Sharding: 8 cores = (batch 4) x (query halves 2). Each core owns 1024 query
rows of one batch element. k is recomputed for the full sequence on both
cores of a pair (computing the remote k half locally costs ~27us of TensorE
and replaces an AllGather that would serialize behind the v exchange on the
collective cores for ~120us). v is computed for the own half only and
exchanged as TWO column-half pair-wise AllGathers: the first is dispatched
mid-way through phase V (~23us in), the second at the end of phase V, so
they pipeline on the collective cores and complete (~100us / ~168us) before
the PV pass that consumes each half (~168us / ~196us) -- the exchange is
fully hidden behind the k/q/scores matmul stream.

SPMD symmetry: the program must not depend on the core's rank, so the host
supplies both x[b].T in global order (xt, for the k projection) and the
core's own query columns (xq, for the q and v-own projections). The v
exchange works in global row order (each core's own rows land at their
global position in the AllGather output), which keeps the j-order of
scores and PV consistent across the pair without rank-dependent addressing.

All matmul operands are fp16 (same TensorE throughput as bf16 on TRN2 --
both upconvert to FP22 in the PE -- but 10 mantissa bits instead of 7:
measured rel-absmax error 4e-4 vs 3.9e-3 for bf16). Accumulation is fp32
in PSUM. Softmax max-subtraction is unnecessary (|scores/32| < ~2.6 by
construction), so ScalarE applies exp(scores/32) directly out of PSUM.
The per-query denominators accumulate into one [i-part, 8] PSUM bank via
N=1 matmuls against a ones column, emitted one jt behind the scores
stream (single accumulation group: one bank-clearing start, per-element
has_written handles per-column accumulation), so a single reciprocal
yields all drain scales and the PV tail has no denominator dependency.

Each tensor lives in ONE wide SBUF tile loaded by ONE 3D-access-pattern
DMA (descriptor generation serializes ~0.6us per DMA instruction on the
shared HWDGE, so instruction count is what matters, not transfer split).
The w ring (bufs=2) carries wv -> wk -> wq -> va -> vb: each allocation's
DMA fires when the buffer two allocations back is released, which both
prefetches the next phase's operand under the current phase's matmuls and
lets the gathered v reuse the dead weight space.

Per-core TensorE work: 1024 N=512 fp16 matmuls (~218us of streaming at
2.4 GHz) + 128 N=1 denominator matmuls; drains run on ScalarE/VectorE
under the matmul stream; the AllGather and all DMA overlap compute.
"""

import numpy as np
from contextlib import ExitStack

import concourse.bacc as bacc
import concourse.tile as tile
import concourse.mybir as mybir

F16 = mybir.dt.float16
F32 = mybir.dt.float32
P = 128
B, S, D = 4, 2048, 1024
SQ = S // 2   # query rows per core
N_CORES = 8
ET = D // P   # contraction tiles over embed dim (projections)
FT = D // P   # feature tiles
JT = S // P   # kv-sequence tiles
IT = SQ // P  # query tiles
NCH = 512     # moving-operand chunk (one fp32 PSUM bank)
INV_SQRT_D = 1.0 / 32.0

_CACHE: dict = {}


def _g3(dram_ap, cols=None):
    """[G*128, C] DRAM slice -> [128, G, C] access pattern (rows = g*128+p)."""
    if cols is not None:
        dram_ap = dram_ap[:, cols[0]:cols[1]]
    return dram_ap.rearrange("(g p) c -> p g c", p=P)


def _s3(tile_ap, width, cols=None):
    """[128, G*width] SBUF tile view -> [128, G, C] matching _g3."""
    v = tile_ap[:].rearrange("p (g c) -> p g c", c=width)
    if cols is not None:
        v = v[:, :, cols[0]:cols[1]]
    return v


def _build(repeats=1):
    nc = bacc.Bacc("TRN2", target_bir_lowering=False, debug=False, num_devices=N_CORES)
    xq = nc.dram_tensor("xq", [D, SQ], F16, kind="ExternalInput").ap()
    xt = nc.dram_tensor("xt", [D, S], F16, kind="ExternalInput").ap()
    wq = nc.dram_tensor("wq", [D, D], F16, kind="ExternalInput").ap()
    wk = nc.dram_tensor("wk", [D, D], F16, kind="ExternalInput").ap()
    wv = nc.dram_tensor("wv", [D, D], F16, kind="ExternalInput").ap()
    out = nc.dram_tensor("out", [SQ, D], F32, kind="ExternalOutput").ap()

    with tile.TileContext(nc) as tc, ExitStack() as ctx:
        x_pool = ctx.enter_context(tc.tile_pool(name="x", bufs=1))
        w_pool = ctx.enter_context(tc.tile_pool(name="w", bufs=1))
        qt_pool = ctx.enter_context(tc.tile_pool(name="qt", bufs=1))
        kt_pool = ctx.enter_context(tc.tile_pool(name="kt", bufs=1))
        exp_pool = ctx.enter_context(tc.tile_pool(name="expT", bufs=1))
        stage_pool = ctx.enter_context(tc.tile_pool(name="stage", bufs=1))
        small_pool = ctx.enter_context(tc.tile_pool(name="small", bufs=1))
        mm_psum = ctx.enter_context(tc.tile_pool(name="mmps", bufs=7, space="PSUM"))
        dn_psum = ctx.enter_context(tc.tile_pool(name="dnps", bufs=1, space="PSUM"))
        dram_pool = ctx.enter_context(tc.tile_pool(name="dram", bufs=1, space="DRAM"))

        xq_t = x_pool.tile([P, ET * SQ], F16, name="xq_t")
        xt_t = x_pool.tile([P, ET * S], F16, name="xt_t")

        def ring(name):
            return w_pool.tile([P, ET * D], F16, name=name, tag="wring", bufs=2)

        wv_t = ring("wv_t")
        wk_t = ring("wk_t")
        # DMA issue order is the service order on the shared DMA engines:
        # front-load what phase V's first chains need (xq j-slice 0 + the
        # first wv column half), then the rest lands under compute.
        nc.sync.dma_start(_s3(xq_t, SQ, (0, P)), _g3(xq, (0, P)))
        nc.sync.dma_start(_s3(wv_t, D, (0, NCH)), _g3(wv, (0, NCH)))
        nc.sync.dma_start(_s3(xq_t, SQ, (P, NCH)), _g3(xq, (P, NCH)))
        nc.sync.dma_start(_s3(xq_t, SQ, (NCH, SQ)), _g3(xq, (NCH, SQ)))
        nc.sync.dma_start(_s3(wv_t, D, (NCH, D)), _g3(wv, (NCH, D)))
        nc.sync.dma_start(_s3(wk_t, D), _g3(wk))
        nc.sync.dma_start(_s3(xt_t, S, (0, NCH)), _g3(xt, (0, NCH)))
        nc.sync.dma_start(_s3(xt_t, S, (NCH, S)), _g3(xt, (NCH, S)))

        for _rep in range(repeats):
            _compute(nc, tc, xq_t, xt_t, wv_t, wk_t, ring, wq, out,
                     qt_pool, kt_pool, exp_pool, stage_pool, small_pool,
                     mm_psum, dn_psum, dram_pool)

    nc.compile()
    return nc


def _compute(nc, tc, xq_t, xt_t, wv_t, wk_t, ring, wq, out,
             qt_pool, kt_pool, exp_pool, stage_pool, small_pool,
             mm_psum, dn_psum, dram_pool):
    groups = [[0, 1], [2, 3], [4, 5], [6, 7]]
    # v is exchanged in two column-half AllGathers: the fc0 half is staged
    # and dispatched mid-way through phase V, so the two collectives
    # pipeline on the collective cores and both complete long before the
    # PV pass that consumes them.
    kv_in = [dram_pool.tile([SQ, NCH], F16, name=f"kv_in{fc}")
             for fc in range(D // NCH)]
    kv_out = [dram_pool.tile([S, NCH], F16, name=f"kv_out{fc}")
              for fc in range(D // NCH)]

    def xqs(et, a, b):
        return xq_t[:, et * SQ + a:et * SQ + b]

    def xts(et, a, b):
        return xt_t[:, et * S + a:et * S + b]

    def ws(w_t, et, a, b):
        return w_t[:, et * D + a:et * D + b]

    # A short burst of throwaway matmuls while the first input slabs are
    # still in flight: costs nothing (PE would be idle) and pays the PE
    # p-state/HAM warm-up ramp before the real stream begins.
    # The warm-up matmuls borrow the denominator pool's bank: the later
    # denominator group opens with start=True, which clears the bank, so
    # the junk it leaves behind is harmless and mm_psum keeps 7 banks.
    warm = small_pool.tile([P, NCH], F16, name="warm")
    nc.vector.memset(warm[:], 0.0)
    psw = dn_psum.tile([P, NCH], F32, name="ps_w", tag="dn")
    for _ in range(7):
        nc.tensor.matmul(psw[:], warm[:, 0:P], warm[:], start=True, stop=True)

    # ---- Phase V: v-own[j_own, f] = x_own @ Wv.T, staged to DRAM for the
    # AllGather. Own rows land at their global position on both cores, so
    # kv_out is in global j-order. fc-outer so the first pass only needs
    # the first wv column half.
    vstage = [stage_pool.tile([P, IT * NCH], F16, name=f"vstage{fc}")
              for fc in range(D // NCH)]
    for fc in range(D // NCH):
        for jq in range(SQ // P):
            ps = mm_psum.tile([P, NCH], F32, name="ps_v", tag="mm")
            for et in range(ET):
                nc.tensor.matmul(
                    ps[:],
                    xqs(et, jq * P, (jq + 1) * P),
                    ws(wv_t, et, fc * NCH, (fc + 1) * NCH),
                    start=(et == 0),
                    stop=(et == ET - 1),
                )
            nc.scalar.activation(
                vstage[fc][:, jq * NCH:(jq + 1) * NCH], ps[:],
                mybir.ActivationFunctionType.Copy)
        nc.sync.dma_start(_g3(kv_in[fc].opt()), _s3(vstage[fc], NCH))
        nc.gpsimd.collective_compute(
            "AllGather", mybir.AluOpType.bypass, replica_groups=groups,
            ins=[kv_in[fc].opt()], outs=[kv_out[fc].opt()],
        )

    wq_t = ring("wq_t")  # ring slot frees at end of phase V; loads during K
    nc.sync.dma_start(_s3(wq_t, D), _g3(wq))

    # ---- Phase K: kT[f, j] = (x @ Wk.T).T for the FULL sequence (recomputed
    # locally instead of a second, serialized AllGather).
    kt_t = kt_pool.tile([P, FT * S], F16, name="kt_t")
    for ft in range(FT):
        for jc in range(S // NCH):
            ps = mm_psum.tile([P, NCH], F32, name="ps_k", tag="mm")
            for et in range(ET):
                nc.tensor.matmul(
                    ps[:],
                    ws(wk_t, et, ft * P, (ft + 1) * P),
                    xts(et, jc * NCH, (jc + 1) * NCH),
                    start=(et == 0),
                    stop=(et == ET - 1),
                )
            dst = kt_t[:, ft * S + jc * NCH:ft * S + (jc + 1) * NCH]
            if jc % 2 == 0:
                nc.vector.tensor_copy(dst, ps[:])
            else:
                nc.scalar.activation(dst, ps[:],
                                     mybir.ActivationFunctionType.Copy)

    # v reuses the ring: va (the fc0 column half, [j, 16 jt x 512]) evicts
    # wv (released end of V), vb evicts wk (released end of K); the DMAs
    # additionally wait on their AllGather's output.
    va_t = ring("va_t")
    nc.sync.dma_start(_s3(va_t, NCH), _g3(kv_out[0]))

    # ---- Phase Q: qT[f, i] for the own query half.
    qt_t = qt_pool.tile([P, FT * SQ], F16, name="qt_t")
    for ft in range(FT):
        for ic in range(SQ // NCH):
            ps = mm_psum.tile([P, NCH], F32, name="ps_q", tag="mm")
            for et in range(ET):
                nc.tensor.matmul(
                    ps[:],
                    ws(wq_t, et, ft * P, (ft + 1) * P),
                    xqs(et, ic * NCH, (ic + 1) * NCH),
                    start=(et == 0),
                    stop=(et == ET - 1),
                )
            qdst = qt_t[:, ft * SQ + ic * NCH:ft * SQ + (ic + 1) * NCH]
            if ic % 2 == 0:
                nc.vector.tensor_copy(qdst, ps[:])
            else:
                nc.scalar.activation(qdst, ps[:],
                                     mybir.ActivationFunctionType.Copy)

    vb_t = ring("vb_t")
    nc.sync.dma_start(_s3(vb_t, NCH), _g3(kv_out[1]))

    def v_sl(fc, jt):
        t = va_t if fc == 0 else vb_t
        return t[:, jt * NCH:(jt + 1) * NCH]

    # ---- Phase S: expT[j, i] = exp(kT.T @ qT / 32), with the softmax
    # denominators accumulating into one [i-part, 8] PSUM bank via N=1
    # matmuls, one jt behind the scores stream.
    ones_t = small_pool.tile([P, 16], F16, name="ones")
    nc.vector.memset(ones_t[:], 1.0)
    ones_f16 = ones_t[:, 0:1]
    recipT = small_pool.tile([P, IT], F32, name="recipT")
    psd = dn_psum.tile([P, IT], F32, name="ps_d", tag="dn")
    exp_t = exp_pool.tile([P, JT * SQ], F16, name="exp_t")

    def exp_sl(jt, a, b):
        return exp_t[:, jt * SQ + a:jt * SQ + b]

    def emit_denoms(jt):
        for it in range(IT):
            nc.tensor.matmul(
                psd[:, it:it + 1],
                exp_sl(jt, it * P, (it + 1) * P),
                ones_f16,
                start=(jt == 0 and it == 0),
                stop=(jt == JT - 1 and it == IT - 1),
            )

    for jt in range(JT):
        for ic in range(SQ // NCH):
            ps = mm_psum.tile([P, NCH], F32, name="ps_s", tag="mm")
            for ft in range(FT):
                nc.tensor.matmul(
                    ps[:],
                    kt_t[:, ft * S + jt * P:ft * S + (jt + 1) * P],
                    qt_t[:, ft * SQ + ic * NCH:ft * SQ + (ic + 1) * NCH],
                    start=(ft == 0),
                    stop=(ft == FT - 1),
                )
            nc.scalar.activation(
                exp_sl(jt, ic * NCH, (ic + 1) * NCH),
                ps[:],
                mybir.ActivationFunctionType.Exp,
                scale=INV_SQRT_D,
            )
        if jt > 0:
            emit_denoms(jt - 1)

    # ---- Phase PV: out[i, f] = (expT.T @ v) * recip[i], normalization
    # folded into the drain as a per-partition scale. Two passes, one per
    # v column half, so pass A only needs the first AllGather's output.
    # The last jt's denominators (whose exp drain is still in flight at
    # the end of phase S) slot in behind the first matmul group; the
    # reciprocal only gates the first drain, not the matmul stream.
    for fc in range(D // NCH):
        for it in range(IT):
            ps = mm_psum.tile([P, NCH], F32, name=f"ps_o{fc}", tag="mm")
            for jt in range(JT):
                nc.tensor.matmul(ps[:], exp_sl(jt, it * P, (it + 1) * P),
                                 v_sl(fc, jt),
                                 start=(jt == 0), stop=(jt == JT - 1))
            if fc == 0 and it == 0:
                emit_denoms(JT - 1)
                nc.vector.reciprocal(recipT[:], psd[:])
            ost = stage_pool.tile([P, NCH], F32, name="ostage", tag="ost", bufs=4)
            if it % 2 == 0:
                nc.scalar.activation(
                    ost[:],
                    ps[:],
                    mybir.ActivationFunctionType.Copy,
                    scale=recipT[:, it:it + 1],
                )
            else:
                nc.vector.tensor_scalar_mul(ost[:], ps[:], recipT[:, it:it + 1])
            nc.sync.dma_start(
                out[it * P:(it + 1) * P, fc * NCH:(fc + 1) * NCH], ost[:])


def _get_nc(repeats=1):
    key = ("nc", repeats)
    if key not in _CACHE:
        _CACHE[key] = _build(repeats)
    return _CACHE[key]


def _prep_inputs(x, Wq, Wk, Wv):
    f16 = np.float16
    x = np.asarray(x, dtype=np.float32)
    wq_t = np.ascontiguousarray(np.asarray(Wq, dtype=np.float32).T.astype(f16))
    wk_t = np.ascontiguousarray(np.asarray(Wk, dtype=np.float32).T.astype(f16))
    wv_t = np.ascontiguousarray(np.asarray(Wv, dtype=np.float32).T.astype(f16))
    xt_b = [np.ascontiguousarray(x[b].T.astype(f16)) for b in range(B)]
    in_maps = []
    for c in range(N_CORES):
        b, h = divmod(c, 2)
        xq_c = np.ascontiguousarray(x[b][h * SQ:(h + 1) * SQ].T.astype(f16))
        in_maps.append({"xq": xq_c, "xt": xt_b[b],
                        "wq": wq_t, "wk": wk_t, "wv": wv_t})
    return in_maps


def _get_runner():
    """Cached jitted dispatcher: one XLA/NEFF compile per process, reused
    across kernel() calls (run_bass_kernel_spmd would recompile per call)."""
    if "runner" in _CACHE:
        return _CACHE["runner"]
    import jax
    from jax.sharding import Mesh, PartitionSpec
    from jax.experimental.shard_map import shard_map
    from concourse.bass2jax import (
        _bass_exec_p, install_neuronx_cc_hook, partition_id_tensor)

    nc = _get_nc()
    install_neuronx_cc_hook()

    in_names, out_names, out_avals = [], [], []
    partition_name = nc.partition_id_tensor.name if nc.partition_id_tensor else None
    for alloc in nc.m.functions[0].allocations:
        if not isinstance(alloc, mybir.MemoryLocationSet):
            continue
        name = alloc.memorylocations[0].name
        if alloc.kind == "ExternalInput":
            if name != partition_name:
                in_names.append(name)
        elif alloc.kind == "ExternalOutput":
            out_names.append(name)
            out_avals.append(jax.core.ShapedArray(
                tuple(alloc.tensor_shape), mybir.dt.np(alloc.dtype)))
    n_params = len(in_names)
    all_names = list(in_names) + out_names
    if partition_name is not None:
        all_names.append(partition_name)

    def _body(*args):
        operands = list(args)
        if partition_name is not None:
            operands.append(partition_id_tensor())
        return tuple(_bass_exec_p.bind(
            *operands,
            out_avals=tuple(out_avals),
            in_names=tuple(all_names),
            out_names=tuple(out_names),
            lowering_input_output_aliases=(),
            sim_require_finite=True,
            sim_require_nnan=True,
            nc=nc,
        ))

    devices = jax.devices()[:N_CORES]
    mesh = Mesh(np.asarray(devices), ("core",))
    nspecs = (PartitionSpec("core"),) * (n_params + len(out_names))
    sharded = jax.jit(
        shard_map(_body, mesh=mesh, in_specs=nspecs,
                  out_specs=(PartitionSpec("core"),) * len(out_names),
                  check_rep=False),
        keep_unused=True,
    )

    def run(in_maps):
        concat_in = [
            np.concatenate([in_maps[c][name] for c in range(N_CORES)], axis=0)
            for name in in_names
        ]
        concat_zero = [
            np.zeros((N_CORES * a.shape[0], *a.shape[1:]), a.dtype)
            for a in out_avals
        ]
        outs = sharded(*concat_in, *concat_zero)
        return {
            name: np.asarray(outs[i]).reshape(N_CORES, *out_avals[i].shape)
            for i, name in enumerate(out_names)
        }

    _CACHE["runner"] = run
    return run


def kernel(x, Wq, Wk, Wv):
    in_maps = _prep_inputs(x, Wq, Wk, Wv)
    res = _get_runner()(in_maps)
    out = np.empty((B, S, D), dtype=np.float32)
    for c in range(N_CORES):
        b, h = divmod(c, 2)
        out[b, h * SQ:(h + 1) * SQ, :] = res["out"][c]
    return out


# revision 31
# speedup vs baseline: 1.0027x; 1.0027x over previous
"""Trainium2 Bass kernel: single-head self-attention.

Reference computation (fp32):
    q = x @ Wq.T ; k = x @ Wk.T ; v = x @ Wv.T        (x: [4, 2048, 1024])
    out = softmax((q @ k.T) / 32) @ v                 ([4, 2048, 1024])

Sharding: 8 cores = (batch 4) x (query halves 2). Each core owns 1024 query
rows of one batch element. k is recomputed for the full sequence on both
cores of a pair (computing the remote k half locally costs ~27us of TensorE
and replaces an AllGather that would serialize behind the v exchange on the
collective cores for ~120us). v is computed for the own half only and
exchanged as TWO column-half pair-wise AllGathers: the first is dispatched
mid-way through phase V (~23us in), the second at the end of phase V, so
they pipeline on the collective cores and complete (~100us / ~168us) before
the PV pass that consumes each half (~168us / ~196us) -- the exchange is
fully hidden behind the k/q/scores matmul stream.

SPMD symmetry: the program must not depend on the core's rank, so the host
supplies both x[b].T in global order (xt, for the k projection) and the
core's own query columns (xq, for the q and v-own projections). The v
exchange works in global row order (each core's own rows land at their
global position in the AllGather output), which keeps the j-order of
scores and PV consistent across the pair without rank-dependent addressing.

All matmul operands are fp16 (same TensorE throughput as bf16 on TRN2 --
both upconvert to FP22 in the PE -- but 10 mantissa bits instead of 7:
measured rel-absmax error 4e-4 vs 3.9e-3 for bf16). Accumulation is fp32
in PSUM. Softmax max-subtraction is unnecessary (|scores/32| < ~2.6 by
construction), so ScalarE applies exp(scores/32) directly out of PSUM.
The per-query denominators accumulate into one [i-part, 8] PSUM bank via
N=1 matmuls against a ones column, emitted four jt behind the scores
stream (single accumulation group: one bank-clearing start, per-element
has_written handles per-column accumulation; the lag keeps them out of
the PE's 4-deep wait queue, which would otherwise block the sequencer on
the exp drains they read), so a single reciprocal yields all drain
scales and the PV tail has no denominator dependency.

Each tensor lives in ONE wide SBUF tile loaded by ONE 3D-access-pattern
DMA (descriptor generation serializes ~0.6us per DMA instruction on the
shared HWDGE, so instruction count is what matters, not transfer split).
The w ring (bufs=2) carries wv -> wk -> wq -> va -> vb: each allocation's
DMA fires when the buffer two allocations back is released, which both
prefetches the next phase's operand under the current phase's matmuls and
lets the gathered v reuse the dead weight space.

Per-core TensorE work: ~218us of N<=512 fp16 matmul streaming at 2.4 GHz
plus 128 N=1 denominator matmuls; drains run on ScalarE/VectorE under the
matmul stream; both AllGathers and all DMA overlap compute. Cost-model
(TimelineSim) total ~230us vs 372us for the previous two-AllGather bf16
variant, with the matmul stream within ~4% of its 218us floor.
"""

import numpy as np
from contextlib import ExitStack

import concourse.bacc as bacc
import concourse.tile as tile
import concourse.mybir as mybir

F16 = mybir.dt.float16
F32 = mybir.dt.float32
P = 128
B, S, D = 4, 2048, 1024
SQ = S // 2   # query rows per core
N_CORES = 8
ET = D // P   # contraction tiles over embed dim (projections)
FT = D // P   # feature tiles
JT = S // P   # kv-sequence tiles
IT = SQ // P  # query tiles
NCH = 512     # moving-operand chunk (one fp32 PSUM bank)
INV_SQRT_D = 1.0 / 32.0

_CACHE: dict = {}


def _g3(dram_ap, cols=None):
    """[G*128, C] DRAM slice -> [128, G, C] access pattern (rows = g*128+p)."""
    if cols is not None:
        dram_ap = dram_ap[:, cols[0]:cols[1]]
    return dram_ap.rearrange("(g p) c -> p g c", p=P)


def _s3(tile_ap, width, cols=None):
    """[128, G*width] SBUF tile view -> [128, G, C] matching _g3."""
    v = tile_ap[:].rearrange("p (g c) -> p g c", c=width)
    if cols is not None:
        v = v[:, :, cols[0]:cols[1]]
    return v


def _build(repeats=1):
    nc = bacc.Bacc("TRN2", target_bir_lowering=False, debug=False, num_devices=N_CORES)
    xq = nc.dram_tensor("xq", [D, SQ], F16, kind="ExternalInput").ap()
    xt = nc.dram_tensor("xt", [D, S], F16, kind="ExternalInput").ap()
    wq = nc.dram_tensor("wq", [D, D], F16, kind="ExternalInput").ap()
    wk = nc.dram_tensor("wk", [D, D], F16, kind="ExternalInput").ap()
    wv = nc.dram_tensor("wv", [D, D], F16, kind="ExternalInput").ap()
    out = nc.dram_tensor("out", [SQ, D], F32, kind="ExternalOutput").ap()

    with tile.TileContext(nc) as tc, ExitStack() as ctx:
        x_pool = ctx.enter_context(tc.tile_pool(name="x", bufs=1))
        w_pool = ctx.enter_context(tc.tile_pool(name="w", bufs=1))
        qt_pool = ctx.enter_context(tc.tile_pool(name="qt", bufs=1))
        kt_pool = ctx.enter_context(tc.tile_pool(name="kt", bufs=1))
        exp_pool = ctx.enter_context(tc.tile_pool(name="expT", bufs=1))
        stage_pool = ctx.enter_context(tc.tile_pool(name="stage", bufs=1))
        small_pool = ctx.enter_context(tc.tile_pool(name="small", bufs=1))
        mm_psum = ctx.enter_context(tc.tile_pool(name="mmps", bufs=7, space="PSUM"))
        dn_psum = ctx.enter_context(tc.tile_pool(name="dnps", bufs=1, space="PSUM"))
        dram_pool = ctx.enter_context(tc.tile_pool(name="dram", bufs=1, space="DRAM"))

        xq_t = x_pool.tile([P, ET * SQ], F16, name="xq_t")
        xt_t = x_pool.tile([P, ET * S], F16, name="xt_t")

        def ring(name):
            return w_pool.tile([P, ET * D], F16, name=name, tag="wring", bufs=2)

        wv_t = ring("wv_t")
        wk_t = ring("wk_t")
        # DMA issue order is the service order on the shared DMA engines:
        # front-load what phase V's first chains need (xq j-slice 0 + the
        # first wv column half), then the rest lands under compute.
        nc.sync.dma_start(_s3(xq_t, SQ, (0, P)), _g3(xq, (0, P)))
        nc.sync.dma_start(_s3(wv_t, D, (0, NCH // 2)), _g3(wv, (0, NCH // 2)))
        nc.sync.dma_start(_s3(xq_t, SQ, (P, NCH)), _g3(xq, (P, NCH)))
        nc.sync.dma_start(_s3(wv_t, D, (NCH // 2, NCH)), _g3(wv, (NCH // 2, NCH)))
        nc.sync.dma_start(_s3(xq_t, SQ, (NCH, SQ)), _g3(xq, (NCH, SQ)))
        nc.sync.dma_start(_s3(wv_t, D, (NCH, D)), _g3(wv, (NCH, D)))
        nc.sync.dma_start(_s3(wk_t, D), _g3(wk))
        nc.sync.dma_start(_s3(xt_t, S, (0, NCH)), _g3(xt, (0, NCH)))
        nc.sync.dma_start(_s3(xt_t, S, (NCH, 2 * NCH)), _g3(xt, (NCH, 2 * NCH)))
        nc.sync.dma_start(_s3(xt_t, S, (2 * NCH, S)), _g3(xt, (2 * NCH, S)))

        for _rep in range(repeats):
            _compute(nc, tc, xq_t, xt_t, wv_t, wk_t, ring, wq, out,
                     qt_pool, kt_pool, exp_pool, stage_pool, small_pool,
                     mm_psum, dn_psum, dram_pool)

    nc.compile()
    return nc


def _compute(nc, tc, xq_t, xt_t, wv_t, wk_t, ring, wq, out,
             qt_pool, kt_pool, exp_pool, stage_pool, small_pool,
             mm_psum, dn_psum, dram_pool):
    groups = [[0, 1], [2, 3], [4, 5], [6, 7]]
    # v is exchanged in two column-half AllGathers: the fc0 half is staged
    # and dispatched mid-way through phase V, so the two collectives
    # pipeline on the collective cores and both complete long before the
    # PV pass that consumes them.
    kv_in = [dram_pool.tile([SQ, NCH], F16, name=f"kv_in{fc}")
             for fc in range(D // NCH)]
    kv_out = [dram_pool.tile([S, NCH], F16, name=f"kv_out{fc}")
              for fc in range(D // NCH)]

    def xqs(et, a, b):
        return xq_t[:, et * SQ + a:et * SQ + b]

    def xts(et, a, b):
        return xt_t[:, et * S + a:et * S + b]

    def ws(w_t, et, a, b):
        return w_t[:, et * D + a:et * D + b]

    # A short burst of throwaway matmuls while the first input slabs are
    # still in flight: costs nothing (PE would be idle) and pays the PE
    # p-state/HAM warm-up ramp before the real stream begins.
    # The warm-up matmuls borrow the denominator pool's bank: the later
    # denominator group opens with start=True, which clears the bank, so
    # the junk it leaves behind is harmless and mm_psum keeps 7 banks.
    warm = small_pool.tile([P, NCH], F16, name="warm")
    nc.vector.memset(warm[:], 0.0)
    psw = dn_psum.tile([P, NCH], F32, name="ps_w", tag="dn")
    for _ in range(7):
        nc.tensor.matmul(psw[:], warm[:, 0:P], warm[:], start=True, stop=True)

    # ---- Phase V: v-own[j_own, f] = x_own @ Wv.T, staged to DRAM for the
    # AllGather. Own rows land at their global position on both cores, so
    # kv_out is in global j-order. fc-outer so the first pass only needs
    # the first wv column half.
    vstage = [stage_pool.tile([P, IT * NCH], F16, name=f"vstage{fc}")
              for fc in range(D // NCH)]
    H = NCH // 2
    for fc in range(D // NCH):
        for jq in range(SQ // P):
            ps = mm_psum.tile([P, NCH], F32, name="ps_v", tag="mm")
            if fc == 0 and jq < 3:
                # two half-width column sub-chunks in ONE accumulation group
                # (single bank-clearing start; per-element has_written makes
                # the second sub-chunk's first write an overwrite): the first
                # 8 matmuls only need the first quarter of wv, which lands
                # ~1.5us earlier than the full half.
                for sub in range(2):
                    for et in range(ET):
                        nc.tensor.matmul(
                            ps[:, sub * H:(sub + 1) * H],
                            xqs(et, jq * P, (jq + 1) * P),
                            ws(wv_t, et, sub * H, (sub + 1) * H),
                            start=(sub == 0 and et == 0),
                            stop=(sub == 1 and et == ET - 1),
                        )
            else:
                for et in range(ET):
                    nc.tensor.matmul(
                        ps[:],
                        xqs(et, jq * P, (jq + 1) * P),
                        ws(wv_t, et, fc * NCH, (fc + 1) * NCH),
                        start=(et == 0),
                        stop=(et == ET - 1),
                    )
            nc.scalar.activation(
                vstage[fc][:, jq * NCH:(jq + 1) * NCH], ps[:],
                mybir.ActivationFunctionType.Copy)
        nc.sync.dma_start(_g3(kv_in[fc].opt()), _s3(vstage[fc], NCH))
        nc.gpsimd.collective_compute(
            "AllGather", mybir.AluOpType.bypass, replica_groups=groups,
            ins=[kv_in[fc].opt()], outs=[kv_out[fc].opt()],
        )

    wq_t = ring("wq_t")  # ring slot frees at end of phase V; loads during K
    nc.sync.dma_start(_s3(wq_t, D), _g3(wq))

    # ---- Phase K: kT[f, j] = (x @ Wk.T).T for the FULL sequence (recomputed
    # locally instead of a second, serialized AllGather).
    kt_t = kt_pool.tile([P, FT * S], F16, name="kt_t")
    for ft in range(FT):
        for jc in range(S // NCH):
            ps = mm_psum.tile([P, NCH], F32, name="ps_k", tag="mm")
            for et in range(ET):
                nc.tensor.matmul(
                    ps[:],
                    ws(wk_t, et, ft * P, (ft + 1) * P),
                    xts(et, jc * NCH, (jc + 1) * NCH),
                    start=(et == 0),
                    stop=(et == ET - 1),
                )
            dst = kt_t[:, ft * S + jc * NCH:ft * S + (jc + 1) * NCH]
            if jc % 2 == 0:
                nc.vector.tensor_copy(dst, ps[:])
            else:
                nc.scalar.activation(dst, ps[:],
                                     mybir.ActivationFunctionType.Copy)

    # v reuses the ring: va (the fc0 column half, [j, 16 jt x 512]) evicts
    # wv (released end of V), vb evicts wk (released end of K); the DMAs
    # additionally wait on their AllGather's output.
    va_t = ring("va_t")
    nc.sync.dma_start(_s3(va_t, NCH), _g3(kv_out[0]))

    # ---- Phase Q: qT[f, i] for the own query half.
    qt_t = qt_pool.tile([P, FT * SQ], F16, name="qt_t")
    for ic in range(SQ // NCH):  # ic-outer: phase S consumes the ic0
        for ft in range(FT):     # column half of every ft first
            ps = mm_psum.tile([P, NCH], F32, name="ps_q", tag="mm")
            for et in range(ET):
                nc.tensor.matmul(
                    ps[:],
                    ws(wq_t, et, ft * P, (ft + 1) * P),
                    xqs(et, ic * NCH, (ic + 1) * NCH),
                    start=(et == 0),
                    stop=(et == ET - 1),
                )
            qdst = qt_t[:, ft * SQ + ic * NCH:ft * SQ + (ic + 1) * NCH]
            if ft % 2 == 0:
                nc.vector.tensor_copy(qdst, ps[:])
            else:
                nc.scalar.activation(qdst, ps[:],
                                     mybir.ActivationFunctionType.Copy)

    vb_t = ring("vb_t")
    nc.sync.dma_start(_s3(vb_t, NCH), _g3(kv_out[1]))

    def v_sl(fc, jt):
        t = va_t if fc == 0 else vb_t
        return t[:, jt * NCH:(jt + 1) * NCH]

    # ---- Phase S: expT[j, i] = exp(kT.T @ qT / 32), with the softmax
    # denominators accumulating into one [i-part, 8] PSUM bank via N=1
    # matmuls, one jt behind the scores stream.
    ones_t = small_pool.tile([P, 16], F16, name="ones")
    nc.vector.memset(ones_t[:], 1.0)
    ones_f16 = ones_t[:, 0:1]
    recipT = small_pool.tile([P, IT], F32, name="recipT")
    psd = dn_psum.tile([P, IT], F32, name="ps_d", tag="dn")
    exp_t = exp_pool.tile([P, JT * SQ], F16, name="exp_t")

    def exp_sl(jt, a, b):
        return exp_t[:, jt * SQ + a:jt * SQ + b]

    def emit_denoms(jt):
        for it in range(IT):
            nc.tensor.matmul(
                psd[:, it:it + 1],
                exp_sl(jt, it * P, (it + 1) * P),
                ones_f16,
                start=(jt == 0 and it == 0),
                stop=(jt == JT - 1 and it == IT - 1),
            )

    for jt in range(JT):
        for ic in range(SQ // NCH):
            ps = mm_psum.tile([P, NCH], F32, name="ps_s", tag="mm")
            for ft in range(FT):
                nc.tensor.matmul(
                    ps[:],
                    kt_t[:, ft * S + jt * P:ft * S + (jt + 1) * P],
                    qt_t[:, ft * SQ + ic * NCH:ft * SQ + (ic + 1) * NCH],
                    start=(ft == 0),
                    stop=(ft == FT - 1),
                )
            nc.scalar.activation(
                exp_sl(jt, ic * NCH, (ic + 1) * NCH),
                ps[:],
                mybir.ActivationFunctionType.Exp,
                scale=INV_SQRT_D,
            )
        # lag the denominator matmuls 4 jt behind the scores stream: they
        # read both ic-halves of exp[jt'], and if the second half's drain is
        # still in flight they occupy the PE's 4-deep wait queue and block
        # the sequencer (measured as 2x ~1.1us stalls at lag 2).
        if jt > 3:
            emit_denoms(jt - 4)
    for jtl in range(JT - 4, JT - 1):
        emit_denoms(jtl)

    # ---- Phase PV: out[i, f] = (expT.T @ v) * recip[i], normalization
    # folded into the drain as a per-partition scale. Two passes, one per
    # v column half, so pass A only needs the first AllGather's output.
    # The last jt's denominators (whose exp drain is still in flight at
    # the end of phase S) slot in behind the first matmul group; the
    # reciprocal only gates the first drain, not the matmul stream.
    for fc in range(D // NCH):
        for it in range(IT):
            ps = mm_psum.tile([P, NCH], F32, name=f"ps_o{fc}", tag="mm")
            for jt in range(JT):
                nc.tensor.matmul(ps[:], exp_sl(jt, it * P, (it + 1) * P),
                                 v_sl(fc, jt),
                                 start=(jt == 0), stop=(jt == JT - 1))
            if fc == 0 and it == 0:
                emit_denoms(JT - 1)
                nc.vector.reciprocal(recipT[:], psd[:])
            ost = stage_pool.tile([P, NCH], F32, name="ostage", tag="ost", bufs=4)
            if it % 2 == 0:
                nc.scalar.activation(
                    ost[:],
                    ps[:],
                    mybir.ActivationFunctionType.Copy,
                    scale=recipT[:, it:it + 1],
                )
            else:
                nc.vector.tensor_scalar_mul(ost[:], ps[:], recipT[:, it:it + 1])
            nc.sync.dma_start(
                out[it * P:(it + 1) * P, fc * NCH:(fc + 1) * NCH], ost[:])


def _get_nc(repeats=1):
    key = ("nc", repeats)
    if key not in _CACHE:
        _CACHE[key] = _build(repeats)
    return _CACHE[key]


def _prep_inputs(x, Wq, Wk, Wv):
    f16 = np.float16
    x = np.asarray(x, dtype=np.float32)
    wq_t = np.ascontiguousarray(np.asarray(Wq, dtype=np.float32).T.astype(f16))
    wk_t = np.ascontiguousarray(np.asarray(Wk, dtype=np.float32).T.astype(f16))
    wv_t = np.ascontiguousarray(np.asarray(Wv, dtype=np.float32).T.astype(f16))
    xt_b = [np.ascontiguousarray(x[b].T.astype(f16)) for b in range(B)]
    in_maps = []
    for c in range(N_CORES):
        b, h = divmod(c, 2)
        xq_c = np.ascontiguousarray(x[b][h * SQ:(h + 1) * SQ].T.astype(f16))
        in_maps.append({"xq": xq_c, "xt": xt_b[b],
                        "wq": wq_t, "wk": wk_t, "wv": wv_t})
    return in_maps


def _get_runner():
    """Cached jitted dispatcher: one XLA/NEFF compile per process, reused
    across kernel() calls (run_bass_kernel_spmd would recompile per call)."""
    if "runner" in _CACHE:
        return _CACHE["runner"]
    import jax
    from jax.sharding import Mesh, PartitionSpec
    from jax.experimental.shard_map import shard_map
    from concourse.bass2jax import (
        _bass_exec_p, install_neuronx_cc_hook, partition_id_tensor)

    nc = _get_nc()
    install_neuronx_cc_hook()

    in_names, out_names, out_avals = [], [], []
    partition_name = nc.partition_id_tensor.name if nc.partition_id_tensor else None
    for alloc in nc.m.functions[0].allocations:
        if not isinstance(alloc, mybir.MemoryLocationSet):
            continue
        name = alloc.memorylocations[0].name
        if alloc.kind == "ExternalInput":
            if name != partition_name:
                in_names.append(name)
        elif alloc.kind == "ExternalOutput":
            out_names.append(name)
            out_avals.append(jax.core.ShapedArray(
                tuple(alloc.tensor_shape), mybir.dt.np(alloc.dtype)))
    n_params = len(in_names)
    all_names = list(in_names) + out_names
    if partition_name is not None:
        all_names.append(partition_name)

    def _body(*args):
        operands = list(args)
        if partition_name is not None:
            operands.append(partition_id_tensor())
        return tuple(_bass_exec_p.bind(
            *operands,
            out_avals=tuple(out_avals),
            in_names=tuple(all_names),
            out_names=tuple(out_names),
            lowering_input_output_aliases=(),
            sim_require_finite=True,
            sim_require_nnan=True,
            nc=nc,
        ))

    devices = jax.devices()[:N_CORES]
    mesh = Mesh(np.asarray(devices), ("core",))
    nspecs = (PartitionSpec("core"),) * (n_params + len(out_names))
    sharded = jax.jit(
        shard_map(_body, mesh=mesh, in_specs=nspecs,
                  out_specs=(PartitionSpec("core"),) * len(out_names),
                  check_rep=False),
        keep_unused=True,
    )

    def run(in_maps):
        concat_in = [
            np.concatenate([in_maps[c][name] for c in range(N_CORES)], axis=0)
            for name in in_names
        ]
        concat_zero = [
            np.zeros((N_CORES * a.shape[0], *a.shape[1:]), a.dtype)
            for a in out_avals
        ]
        outs = sharded(*concat_in, *concat_zero)
        return {
            name: np.asarray(outs[i]).reshape(N_CORES, *out_avals[i].shape)
            for i, name in enumerate(out_names)
        }

    _CACHE["runner"] = run
    return run


def kernel(x, Wq, Wk, Wv):
    in_maps = _prep_inputs(x, Wq, Wk, Wv)
    res = _get_runner()(in_maps)
    out = np.empty((B, S, D), dtype=np.float32)
    for c in range(N_CORES):
        b, h = divmod(c, 2)
        out[b, h * SQ:(h + 1) * SQ, :] = res["out"][c]
    return out


# revision 35
# speedup vs baseline: 1.0049x; 1.0022x over previous
"""Trainium2 Bass kernel: single-head self-attention.

Reference computation (fp32):
    q = x @ Wq.T ; k = x @ Wk.T ; v = x @ Wv.T        (x: [4, 2048, 1024])
    out = softmax((q @ k.T) / 32) @ v                 ([4, 2048, 1024])

Sharding: 8 cores = (batch 4) x (query halves 2). Each core owns 1024 query
rows of one batch element. k is recomputed for the full sequence on both
cores of a pair (computing the remote k half locally costs ~27us of TensorE
and replaces an AllGather that would serialize behind the v exchange on the
collective cores for ~120us). v is computed for the own half only and
exchanged as TWO column-half pair-wise AllGathers: the first is dispatched
mid-way through phase V (~23us in), the second at the end of phase V, so
they pipeline on the collective cores and complete (~100us / ~168us) before
the PV pass that consumes each half (~168us / ~196us) -- the exchange is
fully hidden behind the k/q/scores matmul stream.

SPMD symmetry: the program must not depend on the core's rank, so the host
supplies both x[b].T in global order (xt, for the k projection) and the
core's own query columns (xq, for the q and v-own projections). The v
exchange works in global row order (each core's own rows land at their
global position in the AllGather output), which keeps the j-order of
scores and PV consistent across the pair without rank-dependent addressing.

All matmul operands are fp16 (same TensorE throughput as bf16 on TRN2 --
both upconvert to FP22 in the PE -- but 10 mantissa bits instead of 7:
measured rel-absmax error 4e-4 vs 3.9e-3 for bf16). Accumulation is fp32
in PSUM. Softmax max-subtraction is unnecessary (|scores/32| < ~2.6 by
construction), so ScalarE applies exp(scores/32) directly out of PSUM.
The per-query denominators accumulate into one [i-part, 8] PSUM bank via
N=1 matmuls against a ones column, emitted four jt behind the scores
stream (single accumulation group: one bank-clearing start, per-element
has_written handles per-column accumulation; the lag keeps them out of
the PE's 4-deep wait queue, which would otherwise block the sequencer on
the exp drains they read), so a single reciprocal yields all drain
scales and the PV tail has no denominator dependency.

Each tensor lives in ONE wide SBUF tile loaded by ONE 3D-access-pattern
DMA (descriptor generation serializes ~0.6us per DMA instruction on the
shared HWDGE, so instruction count is what matters, not transfer split).
The w ring (bufs=2) carries wv -> wk -> wq -> va -> vb: each allocation's
DMA fires when the buffer two allocations back is released, which both
prefetches the next phase's operand under the current phase's matmuls and
lets the gathered v reuse the dead weight space.

Phase order is V, Q, K, S, PV: Q runs before K so its trailing qt
drains (which every S chunk waits on via engine-progress semaphores)
complete under K's 55us of matmuls, while K->S hands off pipelined
(each S chunk's kt column slices are written early in each ft's jc
sweep). The final PV group runs as two half-column groups so the first
half's drain+store pipeline under the second half's matmuls.

Per-core TensorE work: ~218us of N<=512 fp16 matmul streaming at 2.4 GHz
plus 128 N=1 denominator matmuls; drains run on ScalarE/VectorE under the
matmul stream; both AllGathers and all DMA overlap compute. Cost-model
(TimelineSim) total ~229us vs 372us for the previous two-AllGather bf16
variant: the matmul stream runs gap-free from ~12us (startup is
DMA-bandwidth-bound) to the end, within ~3% of its 218us floor.
"""

import numpy as np
from contextlib import ExitStack

import concourse.bacc as bacc
import concourse.tile as tile
import concourse.mybir as mybir

F16 = mybir.dt.float16
F32 = mybir.dt.float32
P = 128
B, S, D = 4, 2048, 1024
SQ = S // 2   # query rows per core
N_CORES = 8
ET = D // P   # contraction tiles over embed dim (projections)
FT = D // P   # feature tiles
JT = S // P   # kv-sequence tiles
IT = SQ // P  # query tiles
NCH = 512     # moving-operand chunk (one fp32 PSUM bank)
INV_SQRT_D = 1.0 / 32.0

_CACHE: dict = {}


def _g3(dram_ap, cols=None):
    """[G*128, C] DRAM slice -> [128, G, C] access pattern (rows = g*128+p)."""
    if cols is not None:
        dram_ap = dram_ap[:, cols[0]:cols[1]]
    return dram_ap.rearrange("(g p) c -> p g c", p=P)


def _s3(tile_ap, width, cols=None):
    """[128, G*width] SBUF tile view -> [128, G, C] matching _g3."""
    v = tile_ap[:].rearrange("p (g c) -> p g c", c=width)
    if cols is not None:
        v = v[:, :, cols[0]:cols[1]]
    return v


def _build(repeats=1):
    nc = bacc.Bacc("TRN2", target_bir_lowering=False, debug=False, num_devices=N_CORES)
    xq = nc.dram_tensor("xq", [D, SQ], F16, kind="ExternalInput").ap()
    xt = nc.dram_tensor("xt", [D, S], F16, kind="ExternalInput").ap()
    wq = nc.dram_tensor("wq", [D, D], F16, kind="ExternalInput").ap()
    wk = nc.dram_tensor("wk", [D, D], F16, kind="ExternalInput").ap()
    wv = nc.dram_tensor("wv", [D, D], F16, kind="ExternalInput").ap()
    out = nc.dram_tensor("out", [SQ, D], F32, kind="ExternalOutput").ap()

    with tile.TileContext(nc) as tc, ExitStack() as ctx:
        x_pool = ctx.enter_context(tc.tile_pool(name="x", bufs=1))
        w_pool = ctx.enter_context(tc.tile_pool(name="w", bufs=1))
        qt_pool = ctx.enter_context(tc.tile_pool(name="qt", bufs=1))
        kt_pool = ctx.enter_context(tc.tile_pool(name="kt", bufs=1))
        exp_pool = ctx.enter_context(tc.tile_pool(name="expT", bufs=1))
        stage_pool = ctx.enter_context(tc.tile_pool(name="stage", bufs=1))
        small_pool = ctx.enter_context(tc.tile_pool(name="small", bufs=1))
        mm_psum = ctx.enter_context(tc.tile_pool(name="mmps", bufs=7, space="PSUM"))
        dn_psum = ctx.enter_context(tc.tile_pool(name="dnps", bufs=1, space="PSUM"))
        dram_pool = ctx.enter_context(tc.tile_pool(name="dram", bufs=1, space="DRAM"))

        xq_t = x_pool.tile([P, ET * SQ], F16, name="xq_t")
        xt_t = x_pool.tile([P, ET * S], F16, name="xt_t")

        def ring(name):
            return w_pool.tile([P, ET * D], F16, name=name, tag="wring", bufs=2)

        wv_t = ring("wv_t")
        wq_t = ring("wq_t")
        # DMA issue order is the service order on the shared DMA engines:
        # front-load what phase V's first chains need (xq j-slice 0 + the
        # first wv column half), then the rest lands under compute.
        nc.sync.dma_start(_s3(xq_t, SQ, (0, P)), _g3(xq, (0, P)))
        nc.sync.dma_start(_s3(wv_t, D, (0, NCH // 2)), _g3(wv, (0, NCH // 2)))
        nc.sync.dma_start(_s3(xq_t, SQ, (P, NCH)), _g3(xq, (P, NCH)))
        nc.sync.dma_start(_s3(wv_t, D, (NCH // 2, NCH)), _g3(wv, (NCH // 2, NCH)))
        nc.sync.dma_start(_s3(xq_t, SQ, (NCH, SQ)), _g3(xq, (NCH, SQ)))
        nc.sync.dma_start(_s3(wv_t, D, (NCH, D)), _g3(wv, (NCH, D)))
        nc.sync.dma_start(_s3(wq_t, D), _g3(wq))
        nc.sync.dma_start(_s3(xt_t, S, (0, NCH)), _g3(xt, (0, NCH)))
        nc.sync.dma_start(_s3(xt_t, S, (NCH, 2 * NCH)), _g3(xt, (NCH, 2 * NCH)))
        nc.sync.dma_start(_s3(xt_t, S, (2 * NCH, S)), _g3(xt, (2 * NCH, S)))

        for _rep in range(repeats):
            _compute(nc, tc, xq_t, xt_t, wv_t, wq_t, ring, wk, out,
                     qt_pool, kt_pool, exp_pool, stage_pool, small_pool,
                     mm_psum, dn_psum, dram_pool)

    nc.compile()
    return nc


def _compute(nc, tc, xq_t, xt_t, wv_t, wq_t, ring, wk, out,
             qt_pool, kt_pool, exp_pool, stage_pool, small_pool,
             mm_psum, dn_psum, dram_pool):
    groups = [[0, 1], [2, 3], [4, 5], [6, 7]]
    # v is exchanged in two column-half AllGathers: the fc0 half is staged
    # and dispatched mid-way through phase V, so the two collectives
    # pipeline on the collective cores and both complete long before the
    # PV pass that consumes them.
    kv_in = [dram_pool.tile([SQ, NCH], F16, name=f"kv_in{fc}")
             for fc in range(D // NCH)]
    kv_out = [dram_pool.tile([S, NCH], F16, name=f"kv_out{fc}")
              for fc in range(D // NCH)]

    def xqs(et, a, b):
        return xq_t[:, et * SQ + a:et * SQ + b]

    def xts(et, a, b):
        return xt_t[:, et * S + a:et * S + b]

    def ws(w_t, et, a, b):
        return w_t[:, et * D + a:et * D + b]

    # A short burst of throwaway matmuls while the first input slabs are
    # still in flight: costs nothing (PE would be idle) and pays the PE
    # p-state/HAM warm-up ramp before the real stream begins.
    # The warm-up matmuls borrow the denominator pool's bank: the later
    # denominator group opens with start=True, which clears the bank, so
    # the junk it leaves behind is harmless and mm_psum keeps 7 banks.
    warm = small_pool.tile([P, NCH], F16, name="warm")
    nc.vector.memset(warm[:], 0.0)
    psw = dn_psum.tile([P, NCH], F32, name="ps_w", tag="dn")
    for _ in range(7):
        nc.tensor.matmul(psw[:], warm[:, 0:P], warm[:], start=True, stop=True)

    # ---- Phase V: v-own[j_own, f] = x_own @ Wv.T, staged to DRAM for the
    # AllGather. Own rows land at their global position on both cores, so
    # kv_out is in global j-order. fc-outer so the first pass only needs
    # the first wv column half.
    vstage = [stage_pool.tile([P, IT * NCH], F16, name=f"vstage{fc}")
              for fc in range(D // NCH)]
    H = NCH // 2
    for fc in range(D // NCH):
        for jq in range(SQ // P):
            ps = mm_psum.tile([P, NCH], F32, name="ps_v", tag="mm")
            if fc == 0 and jq < 3:
                # two half-width column sub-chunks in ONE accumulation group
                # (single bank-clearing start; per-element has_written makes
                # the second sub-chunk's first write an overwrite): the first
                # 8 matmuls only need the first quarter of wv, which lands
                # ~1.5us earlier than the full half.
                for sub in range(2):
                    for et in range(ET):
                        nc.tensor.matmul(
                            ps[:, sub * H:(sub + 1) * H],
                            xqs(et, jq * P, (jq + 1) * P),
                            ws(wv_t, et, sub * H, (sub + 1) * H),
                            start=(sub == 0 and et == 0),
                            stop=(sub == 1 and et == ET - 1),
                        )
            else:
                for et in range(ET):
                    nc.tensor.matmul(
                        ps[:],
                        xqs(et, jq * P, (jq + 1) * P),
                        ws(wv_t, et, fc * NCH, (fc + 1) * NCH),
                        start=(et == 0),
                        stop=(et == ET - 1),
                    )
            nc.scalar.activation(
                vstage[fc][:, jq * NCH:(jq + 1) * NCH], ps[:],
                mybir.ActivationFunctionType.Copy)
        nc.sync.dma_start(_g3(kv_in[fc].opt()), _s3(vstage[fc], NCH))
        nc.gpsimd.collective_compute(
            "AllGather", mybir.AluOpType.bypass, replica_groups=groups,
            ins=[kv_in[fc].opt()], outs=[kv_out[fc].opt()],
        )

    wk_t = ring("wk_t")  # ring slot frees at end of phase V; loads during Q
    nc.sync.dma_start(_s3(wk_t, D), _g3(wk))

    # ---- Phase Q: qT[f, i] for the own query half. Q runs BEFORE K so its
    # trailing drains (which phase S waits on via engine-progress
    # semaphores, since every S chunk reads a full column half of qt)
    # complete under K's 55us of matmuls instead of right at S's entry.
    qt_t = qt_pool.tile([P, FT * SQ], F16, name="qt_t")
    for ic in range(SQ // NCH):
        for ft in range(FT):
            ps = mm_psum.tile([P, NCH], F32, name="ps_q", tag="mm")
            for et in range(ET):
                nc.tensor.matmul(
                    ps[:],
                    ws(wq_t, et, ft * P, (ft + 1) * P),
                    xqs(et, ic * NCH, (ic + 1) * NCH),
                    start=(et == 0),
                    stop=(et == ET - 1),
                )
            qdst = qt_t[:, ft * SQ + ic * NCH:ft * SQ + (ic + 1) * NCH]
            if ft % 2 == 0:
                nc.vector.tensor_copy(qdst, ps[:])
            else:
                nc.scalar.activation(qdst, ps[:],
                                     mybir.ActivationFunctionType.Copy)

    # v reuses the ring: va evicts wq (released end of Q), vb evicts wk
    # (released end of K); the DMAs additionally wait on their AllGather's
    # output.
    va_t = ring("va_t")
    nc.sync.dma_start(_s3(va_t, NCH), _g3(kv_out[0]))

    # ---- Phase K: kT[f, j] = (x @ Wk.T).T for the FULL sequence (recomputed
    # locally instead of a second, serialized AllGather). K feeds S with an
    # intrinsically pipelined handoff: each S chunk's kt column slices are
    # written early within each ft's jc sweep.
    kt_t = kt_pool.tile([P, FT * S], F16, name="kt_t")
    for ft in range(FT):
        for jc in range(S // NCH):
            ps = mm_psum.tile([P, NCH], F32, name="ps_k", tag="mm")
            for et in range(ET):
                nc.tensor.matmul(
                    ps[:],
                    ws(wk_t, et, ft * P, (ft + 1) * P),
                    xts(et, jc * NCH, (jc + 1) * NCH),
                    start=(et == 0),
                    stop=(et == ET - 1),
                )
            dst = kt_t[:, ft * S + jc * NCH:ft * S + (jc + 1) * NCH]
            if jc % 2 == 0:
                nc.vector.tensor_copy(dst, ps[:])
            else:
                nc.scalar.activation(dst, ps[:],
                                     mybir.ActivationFunctionType.Copy)

    vb_t = ring("vb_t")
    nc.sync.dma_start(_s3(vb_t, NCH), _g3(kv_out[1]))

    def v_sl(fc, jt):
        t = va_t if fc == 0 else vb_t
        return t[:, jt * NCH:(jt + 1) * NCH]

    # ---- Phase S: expT[j, i] = exp(kT.T @ qT / 32), with the softmax
    # denominators accumulating into one [i-part, 8] PSUM bank via N=1
    # matmuls, one jt behind the scores stream.
    ones_t = small_pool.tile([P, 16], F16, name="ones")
    nc.vector.memset(ones_t[:], 1.0)
    ones_f16 = ones_t[:, 0:1]
    recipT = small_pool.tile([P, IT], F32, name="recipT")
    psd = dn_psum.tile([P, IT], F32, name="ps_d", tag="dn")
    exp_t = exp_pool.tile([P, JT * SQ], F16, name="exp_t")

    def exp_sl(jt, a, b):
        return exp_t[:, jt * SQ + a:jt * SQ + b]

    def emit_denoms(jt):
        for it in range(IT):
            nc.tensor.matmul(
                psd[:, it:it + 1],
                exp_sl(jt, it * P, (it + 1) * P),
                ones_f16,
                start=(jt == 0 and it == 0),
                stop=(jt == JT - 1 and it == IT - 1),
            )

    for jt in range(JT):
        for ic in range(SQ // NCH):
            ps = mm_psum.tile([P, NCH], F32, name="ps_s", tag="mm")
            for ft in range(FT):
                nc.tensor.matmul(
                    ps[:],
                    kt_t[:, ft * S + jt * P:ft * S + (jt + 1) * P],
                    qt_t[:, ft * SQ + ic * NCH:ft * SQ + (ic + 1) * NCH],
                    start=(ft == 0),
                    stop=(ft == FT - 1),
                )
            nc.scalar.activation(
                exp_sl(jt, ic * NCH, (ic + 1) * NCH),
                ps[:],
                mybir.ActivationFunctionType.Exp,
                scale=INV_SQRT_D,
            )
        # lag the denominator matmuls 4 jt behind the scores stream: they
        # read both ic-halves of exp[jt'], and if the second half's drain is
        # still in flight they occupy the PE's 4-deep wait queue and block
        # the sequencer (measured as 2x ~1.1us stalls at lag 2).
        if jt > 3:
            emit_denoms(jt - 4)
    for jtl in range(JT - 4, JT - 1):
        emit_denoms(jtl)

    # ---- Phase PV: out[i, f] = (expT.T @ v) * recip[i], normalization
    # folded into the drain as a per-partition scale. Two passes, one per
    # v column half, so pass A only needs the first AllGather's output.
    # The last jt's denominators (whose exp drain is still in flight at
    # the end of phase S) slot in behind the first matmul group; the
    # reciprocal only gates the first drain, not the matmul stream.
    H = NCH // 2
    for fc in range(D // NCH):
        for it in range(IT):
            last_group = fc == D // NCH - 1 and it == IT - 1
            # The final group runs as two half-column groups in separate
            # banks so the first half's drain + store pipeline under the
            # second half's matmuls, shortening the exposed tail chain.
            subs = ((0, H), (H, NCH)) if last_group else ((0, NCH),)
            for a, b in subs:
                ps = mm_psum.tile([P, b - a], F32, name=f"ps_o{fc}", tag="mm")
                for jt in range(JT):
                    nc.tensor.matmul(ps[:], exp_sl(jt, it * P, (it + 1) * P),
                                     v_sl(fc, jt)[:, a:b],
                                     start=(jt == 0), stop=(jt == JT - 1))
                if fc == 0 and it == 0:
                    emit_denoms(JT - 1)
                    nc.vector.reciprocal(recipT[:], psd[:])
                ost = stage_pool.tile([P, b - a], F32, name="ostage",
                                      tag="ost", bufs=4)
                if it % 2 == 0:
                    nc.scalar.activation(
                        ost[:],
                        ps[:],
                        mybir.ActivationFunctionType.Copy,
                        scale=recipT[:, it:it + 1],
                    )
                else:
                    nc.vector.tensor_scalar_mul(ost[:], ps[:],
                                                recipT[:, it:it + 1])
                nc.sync.dma_start(
                    out[it * P:(it + 1) * P, fc * NCH + a:fc * NCH + b],
                    ost[:])


def _get_nc(repeats=1):
    key = ("nc", repeats)
    if key not in _CACHE:
        _CACHE[key] = _build(repeats)
    return _CACHE[key]


def _prep_inputs(x, Wq, Wk, Wv):
    f16 = np.float16
    x = np.asarray(x, dtype=np.float32)
    wq_t = np.ascontiguousarray(np.asarray(Wq, dtype=np.float32).T.astype(f16))
    wk_t = np.ascontiguousarray(np.asarray(Wk, dtype=np.float32).T.astype(f16))
    wv_t = np.ascontiguousarray(np.asarray(Wv, dtype=np.float32).T.astype(f16))
    xt_b = [np.ascontiguousarray(x[b].T.astype(f16)) for b in range(B)]
    in_maps = []
    for c in range(N_CORES):
        b, h = divmod(c, 2)
        xq_c = np.ascontiguousarray(x[b][h * SQ:(h + 1) * SQ].T.astype(f16))
        in_maps.append({"xq": xq_c, "xt": xt_b[b],
                        "wq": wq_t, "wk": wk_t, "wv": wv_t})
    return in_maps


def _get_runner():
    """Cached jitted dispatcher: one XLA/NEFF compile per process, reused
    across kernel() calls (run_bass_kernel_spmd would recompile per call)."""
    if "runner" in _CACHE:
        return _CACHE["runner"]
    import jax
    from jax.sharding import Mesh, PartitionSpec
    from jax.experimental.shard_map import shard_map
    from concourse.bass2jax import (
        _bass_exec_p, install_neuronx_cc_hook, partition_id_tensor)

    nc = _get_nc()
    install_neuronx_cc_hook()

    in_names, out_names, out_avals = [], [], []
    partition_name = nc.partition_id_tensor.name if nc.partition_id_tensor else None
    for alloc in nc.m.functions[0].allocations:
        if not isinstance(alloc, mybir.MemoryLocationSet):
            continue
        name = alloc.memorylocations[0].name
        if alloc.kind == "ExternalInput":
            if name != partition_name:
                in_names.append(name)
        elif alloc.kind == "ExternalOutput":
            out_names.append(name)
            out_avals.append(jax.core.ShapedArray(
                tuple(alloc.tensor_shape), mybir.dt.np(alloc.dtype)))
    n_params = len(in_names)
    all_names = list(in_names) + out_names
    if partition_name is not None:
        all_names.append(partition_name)

    def _body(*args):
        operands = list(args)
        if partition_name is not None:
            operands.append(partition_id_tensor())
        return tuple(_bass_exec_p.bind(
            *operands,
            out_avals=tuple(out_avals),
            in_names=tuple(all_names),
            out_names=tuple(out_names),
            lowering_input_output_aliases=(),
            sim_require_finite=True,
            sim_require_nnan=True,
            nc=nc,
        ))

    devices = jax.devices()[:N_CORES]
    mesh = Mesh(np.asarray(devices), ("core",))
    nspecs = (PartitionSpec("core"),) * (n_params + len(out_names))
    sharded = jax.jit(
        shard_map(_body, mesh=mesh, in_specs=nspecs,
                  out_specs=(PartitionSpec("core"),) * len(out_names),
                  check_rep=False),
        keep_unused=True,
    )

    def run(in_maps):
        concat_in = [
            np.concatenate([in_maps[c][name] for c in range(N_CORES)], axis=0)
            for name in in_names
        ]
        concat_zero = [
            np.zeros((N_CORES * a.shape[0], *a.shape[1:]), a.dtype)
            for a in out_avals
        ]
        outs = sharded(*concat_in, *concat_zero)
        return {
            name: np.asarray(outs[i]).reshape(N_CORES, *out_avals[i].shape)
            for i, name in enumerate(out_names)
        }

    _CACHE["runner"] = run
    return run


def kernel(x, Wq, Wk, Wv):
    in_maps = _prep_inputs(x, Wq, Wk, Wv)
    res = _get_runner()(in_maps)
    out = np.empty((B, S, D), dtype=np.float32)
    for c in range(N_CORES):
        b, h = divmod(c, 2)
        out[b, h * SQ:(h + 1) * SQ, :] = res["out"][c]
    return out


# revision 40
# speedup vs baseline: 1.0093x; 1.0043x over previous
"""Trainium2 Bass kernel: single-head self-attention.

Reference computation (fp32):
    q = x @ Wq.T ; k = x @ Wk.T ; v = x @ Wv.T        (x: [4, 2048, 1024])
    out = softmax((q @ k.T) / 32) @ v                 ([4, 2048, 1024])

Sharding: 8 cores = (batch 4) x (query halves 2). Each core owns 1024 query
rows of one batch element. k is recomputed for the full sequence on both
cores of a pair (computing the remote k half locally costs ~27us of TensorE
and replaces an AllGather that would serialize behind the v exchange on the
collective cores for ~120us). v is computed for the own half only and
exchanged as TWO column-half pair-wise AllGathers: the first is dispatched
mid-way through phase V (~23us in), the second at the end of phase V, so
they pipeline on the collective cores and complete (~100us / ~168us) before
the PV pass that consumes each half (~168us / ~196us) -- the exchange is
fully hidden behind the k/q/scores matmul stream.

SPMD symmetry: the program must not depend on the core's rank, so the host
supplies both x[b].T in global order (xt, for the k projection) and the
core's own query columns (xq, for the q and v-own projections). The v
exchange works in global row order (each core's own rows land at their
global position in the AllGather output), which keeps the j-order of
scores and PV consistent across the pair without rank-dependent addressing.

All matmul operands are fp16 (same TensorE throughput as bf16 on TRN2 --
both upconvert to FP22 in the PE -- but 10 mantissa bits instead of 7:
measured rel-absmax error 4e-4 vs 3.9e-3 for bf16). Accumulation is fp32
in PSUM. Softmax max-subtraction is unnecessary (|scores/32| < ~2.6 by
construction), so ScalarE applies exp(scores/32) directly out of PSUM.
The per-query denominators accumulate into one [i-part, 8] PSUM bank via
N=1 matmuls against a ones column, emitted four jt behind the scores
stream (single accumulation group: one bank-clearing start, per-element
has_written handles per-column accumulation; the lag keeps them out of
the PE's 4-deep wait queue, which would otherwise block the sequencer on
the exp drains they read), so a single reciprocal yields all drain
scales and the PV tail has no denominator dependency.

Each tensor lives in ONE wide SBUF tile loaded by ONE 3D-access-pattern
DMA (descriptor generation serializes ~0.6us per DMA instruction on the
shared HWDGE, so instruction count is what matters, not transfer split).
The w ring (bufs=2) carries wv -> wk -> wq -> va -> vb: each allocation's
DMA fires when the buffer two allocations back is released, which both
prefetches the next phase's operand under the current phase's matmuls and
lets the gathered v reuse the dead weight space.

Phase order is V, Q, K, S, PV: Q runs before K so its trailing qt
drains (which every S chunk waits on via engine-progress semaphores)
complete under K's 55us of matmuls, while K->S hands off pipelined
(each S chunk's kt column slices are written early in each ft's jc
sweep). The final PV group runs as two half-column groups so the first
half's drain+store pipeline under the second half's matmuls.

Per-core TensorE work: ~218us of N<=512 fp16 matmul streaming at 2.4 GHz
plus 128 N=1 denominator matmuls; drains run on ScalarE/VectorE under the
matmul stream; both AllGathers and all DMA overlap compute. Cost-model
(TimelineSim) total ~228us vs 372us for the previous two-AllGather bf16
variant: the matmul stream runs gap-free from ~12us (startup is
DMA-bandwidth-bound) to the end, within ~3% of its 218us floor.
"""

import numpy as np
from contextlib import ExitStack

import concourse.bacc as bacc
import concourse.tile as tile
import concourse.mybir as mybir

F16 = mybir.dt.float16
F32 = mybir.dt.float32
P = 128
B, S, D = 4, 2048, 1024
SQ = S // 2   # query rows per core
N_CORES = 8
ET = D // P   # contraction tiles over embed dim (projections)
FT = D // P   # feature tiles
JT = S // P   # kv-sequence tiles
IT = SQ // P  # query tiles
NCH = 512     # moving-operand chunk (one fp32 PSUM bank)
INV_SQRT_D = 1.0 / 32.0

_CACHE: dict = {}


def _g3(dram_ap, cols=None):
    """[G*128, C] DRAM slice -> [128, G, C] access pattern (rows = g*128+p)."""
    if cols is not None:
        dram_ap = dram_ap[:, cols[0]:cols[1]]
    return dram_ap.rearrange("(g p) c -> p g c", p=P)


def _s3(tile_ap, width, cols=None):
    """[128, G*width] SBUF tile view -> [128, G, C] matching _g3."""
    v = tile_ap[:].rearrange("p (g c) -> p g c", c=width)
    if cols is not None:
        v = v[:, :, cols[0]:cols[1]]
    return v


def _build(repeats=1):
    nc = bacc.Bacc("TRN2", target_bir_lowering=False, debug=False, num_devices=N_CORES)
    xq = nc.dram_tensor("xq", [D, SQ], F16, kind="ExternalInput").ap()
    xt = nc.dram_tensor("xt", [D, S], F16, kind="ExternalInput").ap()
    wq = nc.dram_tensor("wq", [D, D], F16, kind="ExternalInput").ap()
    wk = nc.dram_tensor("wk", [D, D], F16, kind="ExternalInput").ap()
    wv = nc.dram_tensor("wv", [D, D], F16, kind="ExternalInput").ap()
    out = nc.dram_tensor("out", [SQ, D], F32, kind="ExternalOutput").ap()

    with tile.TileContext(nc) as tc, ExitStack() as ctx:
        x_pool = ctx.enter_context(tc.tile_pool(name="x", bufs=1))
        w_pool = ctx.enter_context(tc.tile_pool(name="w", bufs=1))
        qt_pool = ctx.enter_context(tc.tile_pool(name="qt", bufs=1))
        kt_pool = ctx.enter_context(tc.tile_pool(name="kt", bufs=1))
        exp_pool = ctx.enter_context(tc.tile_pool(name="expT", bufs=1))
        stage_pool = ctx.enter_context(tc.tile_pool(name="stage", bufs=1))
        small_pool = ctx.enter_context(tc.tile_pool(name="small", bufs=1))
        mm_psum = ctx.enter_context(tc.tile_pool(name="mmps", bufs=7, space="PSUM"))
        dn_psum = ctx.enter_context(tc.tile_pool(name="dnps", bufs=1, space="PSUM"))
        dram_pool = ctx.enter_context(tc.tile_pool(name="dram", bufs=1, space="DRAM"))

        xq_t = x_pool.tile([P, ET * SQ], F16, name="xq_t")
        xt_t = x_pool.tile([P, ET * S], F16, name="xt_t")

        def ring(name):
            return w_pool.tile([P, ET * D], F16, name=name, tag="wring", bufs=2)

        wv_t = ring("wv_t")
        wq_t = ring("wq_t")
        # DMA issue order is the service order on the shared DMA engines:
        # front-load what phase V's first chains need (xq j-slice 0 + the
        # first wv column half), then the rest lands under compute.
        nc.sync.dma_start(_s3(xq_t, SQ, (0, 2 * P)), _g3(xq, (0, 2 * P)))
        nc.sync.dma_start(_s3(wv_t, D, (0, NCH // 2)), _g3(wv, (0, NCH // 2)))
        nc.sync.dma_start(_s3(xq_t, SQ, (2 * P, NCH)), _g3(xq, (2 * P, NCH)))
        nc.sync.dma_start(_s3(wv_t, D, (NCH // 2, NCH)), _g3(wv, (NCH // 2, NCH)))
        nc.sync.dma_start(_s3(xq_t, SQ, (NCH, SQ)), _g3(xq, (NCH, SQ)))
        nc.sync.dma_start(_s3(wv_t, D, (NCH, D)), _g3(wv, (NCH, D)))
        nc.sync.dma_start(_s3(wq_t, D), _g3(wq))
        nc.sync.dma_start(_s3(xt_t, S, (0, NCH)), _g3(xt, (0, NCH)))
        nc.sync.dma_start(_s3(xt_t, S, (NCH, 2 * NCH)), _g3(xt, (NCH, 2 * NCH)))
        nc.sync.dma_start(_s3(xt_t, S, (2 * NCH, S)), _g3(xt, (2 * NCH, S)))

        for _rep in range(repeats):
            _compute(nc, tc, xq_t, xt_t, wv_t, wq_t, ring, wk, out,
                     qt_pool, kt_pool, exp_pool, stage_pool, small_pool,
                     mm_psum, dn_psum, dram_pool)

    nc.compile()
    return nc


def _compute(nc, tc, xq_t, xt_t, wv_t, wq_t, ring, wk, out,
             qt_pool, kt_pool, exp_pool, stage_pool, small_pool,
             mm_psum, dn_psum, dram_pool):
    groups = [[0, 1], [2, 3], [4, 5], [6, 7]]
    # v is exchanged in two column-half AllGathers: the fc0 half is staged
    # and dispatched mid-way through phase V, so the two collectives
    # pipeline on the collective cores and both complete long before the
    # PV pass that consumes them.
    kv_in = [dram_pool.tile([SQ, NCH], F16, name=f"kv_in{fc}")
             for fc in range(D // NCH)]
    kv_out = [dram_pool.tile([S, NCH], F16, name=f"kv_out{fc}")
              for fc in range(D // NCH)]

    def xqs(et, a, b):
        return xq_t[:, et * SQ + a:et * SQ + b]

    def xts(et, a, b):
        return xt_t[:, et * S + a:et * S + b]

    def ws(w_t, et, a, b):
        return w_t[:, et * D + a:et * D + b]

    # A short burst of throwaway matmuls while the first input slabs are
    # still in flight: costs nothing (PE would be idle) and pays the PE
    # p-state/HAM warm-up ramp before the real stream begins.
    # The warm-up matmuls borrow the denominator pool's bank: the later
    # denominator group opens with start=True, which clears the bank, so
    # the junk it leaves behind is harmless and mm_psum keeps 7 banks.
    warm = small_pool.tile([P, NCH], F16, name="warm")
    nc.vector.memset(warm[:], 0.0)
    psw = dn_psum.tile([P, NCH], F32, name="ps_w", tag="dn")
    for _ in range(7):
        nc.tensor.matmul(psw[:], warm[:, 0:P], warm[:], start=True, stop=True)

    # ---- Phase V: v-own[j_own, f] = x_own @ Wv.T, staged to DRAM for the
    # AllGather. Own rows land at their global position on both cores, so
    # kv_out is in global j-order. fc-outer so the first pass only needs
    # the first wv column half.
    vstage = [stage_pool.tile([P, IT * NCH], F16, name=f"vstage{fc}")
              for fc in range(D // NCH)]
    # The first three jq chunks run as half-width column sub-chunks, all
    # first-quarter groups before any second-quarter group (one accumulation
    # group per bank: single bank-clearing start, per-element has_written
    # makes the second sub-chunk's first write an overwrite). The PE can
    # then stream three chunks off the first wv quarter + first xq slab
    # while the rest of the input is still in flight.
    H = NCH // 2
    NEARLY = 3
    early = [mm_psum.tile([P, NCH], F32, name="ps_v", tag="mm")
             for _ in range(NEARLY)]
    for sub in range(2):
        for jq in range(NEARLY):
            for et in range(ET):
                nc.tensor.matmul(
                    early[jq][:, sub * H:(sub + 1) * H],
                    xqs(et, jq * P, (jq + 1) * P),
                    ws(wv_t, et, sub * H, (sub + 1) * H),
                    start=(sub == 0 and et == 0),
                    stop=(sub == 1 and et == ET - 1),
                )
    for jq in range(NEARLY):
        nc.scalar.activation(
            vstage[0][:, jq * NCH:(jq + 1) * NCH], early[jq][:],
            mybir.ActivationFunctionType.Copy)
    for fc in range(D // NCH):
        for jq in range(NEARLY if fc == 0 else 0, SQ // P):
            ps = mm_psum.tile([P, NCH], F32, name="ps_v", tag="mm")
            for et in range(ET):
                nc.tensor.matmul(
                    ps[:],
                    xqs(et, jq * P, (jq + 1) * P),
                    ws(wv_t, et, fc * NCH, (fc + 1) * NCH),
                    start=(et == 0),
                    stop=(et == ET - 1),
                )
            nc.scalar.activation(
                vstage[fc][:, jq * NCH:(jq + 1) * NCH], ps[:],
                mybir.ActivationFunctionType.Copy)
        nc.sync.dma_start(_g3(kv_in[fc].opt()), _s3(vstage[fc], NCH))
        nc.gpsimd.collective_compute(
            "AllGather", mybir.AluOpType.bypass, replica_groups=groups,
            ins=[kv_in[fc].opt()], outs=[kv_out[fc].opt()],
        )

    wk_t = ring("wk_t")  # ring slot frees at end of phase V; loads during Q
    nc.sync.dma_start(_s3(wk_t, D), _g3(wk))

    # ---- Phase Q: qT[f, i] for the own query half. Q runs BEFORE K so its
    # trailing drains (which phase S waits on via engine-progress
    # semaphores, since every S chunk reads a full column half of qt)
    # complete under K's 55us of matmuls instead of right at S's entry.
    qt_t = qt_pool.tile([P, FT * SQ], F16, name="qt_t")
    for ic in range(SQ // NCH):
        for ft in range(FT):
            ps = mm_psum.tile([P, NCH], F32, name="ps_q", tag="mm")
            for et in range(ET):
                nc.tensor.matmul(
                    ps[:],
                    ws(wq_t, et, ft * P, (ft + 1) * P),
                    xqs(et, ic * NCH, (ic + 1) * NCH),
                    start=(et == 0),
                    stop=(et == ET - 1),
                )
            qdst = qt_t[:, ft * SQ + ic * NCH:ft * SQ + (ic + 1) * NCH]
            if ft % 2 == 0:
                nc.vector.tensor_copy(qdst, ps[:])
            else:
                nc.scalar.activation(qdst, ps[:],
                                     mybir.ActivationFunctionType.Copy)

    # v reuses the ring: va evicts wq (released end of Q), vb evicts wk
    # (released end of K); the DMAs additionally wait on their AllGather's
    # output.
    va_t = ring("va_t")
    nc.sync.dma_start(_s3(va_t, NCH), _g3(kv_out[0]))

    # ---- Phase K: kT[f, j] = (x @ Wk.T).T for the FULL sequence (recomputed
    # locally instead of a second, serialized AllGather). K feeds S with an
    # intrinsically pipelined handoff: each S chunk's kt column slices are
    # written early within each ft's jc sweep.
    kt_t = kt_pool.tile([P, FT * S], F16, name="kt_t")
    for ft in range(FT):
        for jc in range(S // NCH):
            ps = mm_psum.tile([P, NCH], F32, name="ps_k", tag="mm")
            for et in range(ET):
                nc.tensor.matmul(
                    ps[:],
                    ws(wk_t, et, ft * P, (ft + 1) * P),
                    xts(et, jc * NCH, (jc + 1) * NCH),
                    start=(et == 0),
                    stop=(et == ET - 1),
                )
            dst = kt_t[:, ft * S + jc * NCH:ft * S + (jc + 1) * NCH]
            if jc % 2 == 0:
                nc.vector.tensor_copy(dst, ps[:])
            else:
                nc.scalar.activation(dst, ps[:],
                                     mybir.ActivationFunctionType.Copy)

    vb_t = ring("vb_t")
    nc.sync.dma_start(_s3(vb_t, NCH), _g3(kv_out[1]))

    def v_sl(fc, jt):
        t = va_t if fc == 0 else vb_t
        return t[:, jt * NCH:(jt + 1) * NCH]

    # ---- Phase S: expT[j, i] = exp(kT.T @ qT / 32), with the softmax
    # denominators accumulating into one [i-part, 8] PSUM bank via N=1
    # matmuls, one jt behind the scores stream.
    ones_t = small_pool.tile([P, 16], F16, name="ones")
    nc.vector.memset(ones_t[:], 1.0)
    ones_f16 = ones_t[:, 0:1]
    recipT = small_pool.tile([P, IT], F32, name="recipT")
    psd = dn_psum.tile([P, IT], F32, name="ps_d", tag="dn")
    exp_t = exp_pool.tile([P, JT * SQ], F16, name="exp_t")

    def exp_sl(jt, a, b):
        return exp_t[:, jt * SQ + a:jt * SQ + b]

    def emit_denoms(jt):
        for it in range(IT):
            nc.tensor.matmul(
                psd[:, it:it + 1],
                exp_sl(jt, it * P, (it + 1) * P),
                ones_f16,
                start=(jt == 0 and it == 0),
                stop=(jt == JT - 1 and it == IT - 1),
            )

    for jt in range(JT):
        for ic in range(SQ // NCH):
            ps = mm_psum.tile([P, NCH], F32, name="ps_s", tag="mm")
            for ft in range(FT):
                nc.tensor.matmul(
                    ps[:],
                    kt_t[:, ft * S + jt * P:ft * S + (jt + 1) * P],
                    qt_t[:, ft * SQ + ic * NCH:ft * SQ + (ic + 1) * NCH],
                    start=(ft == 0),
                    stop=(ft == FT - 1),
                )
            nc.scalar.activation(
                exp_sl(jt, ic * NCH, (ic + 1) * NCH),
                ps[:],
                mybir.ActivationFunctionType.Exp,
                scale=INV_SQRT_D,
            )
        # lag the denominator matmuls 4 jt behind the scores stream: they
        # read both ic-halves of exp[jt'], and if the second half's drain is
        # still in flight they occupy the PE's 4-deep wait queue and block
        # the sequencer (measured as 2x ~1.1us stalls at lag 2).
        if jt > 3:
            emit_denoms(jt - 4)
    for jtl in range(JT - 4, JT - 1):
        emit_denoms(jtl)

    # ---- Phase PV: out[i, f] = (expT.T @ v) * recip[i], normalization
    # folded into the drain as a per-partition scale. Two passes, one per
    # v column half, so pass A only needs the first AllGather's output.
    # The last jt's denominators (whose exp drain is still in flight at
    # the end of phase S) slot in behind the first matmul group; the
    # reciprocal only gates the first drain, not the matmul stream.
    H = NCH // 2
    for fc in range(D // NCH):
        for it in range(IT):
            last_group = fc == D // NCH - 1 and it == IT - 1
            # The final group runs as two half-column groups in separate
            # banks so the first half's drain + store pipeline under the
            # second half's matmuls, shortening the exposed tail chain.
            subs = ((0, H), (H, NCH)) if last_group else ((0, NCH),)
            for a, b in subs:
                ps = mm_psum.tile([P, b - a], F32, name=f"ps_o{fc}", tag="mm")
                for jt in range(JT):
                    nc.tensor.matmul(ps[:], exp_sl(jt, it * P, (it + 1) * P),
                                     v_sl(fc, jt)[:, a:b],
                                     start=(jt == 0), stop=(jt == JT - 1))
                if fc == 0 and it == 0:
                    emit_denoms(JT - 1)
                    nc.vector.reciprocal(recipT[:], psd[:])
                ost = stage_pool.tile([P, b - a], F32, name="ostage",
                                      tag="ost", bufs=4)
                if it % 2 == 0:
                    nc.scalar.activation(
                        ost[:],
                        ps[:],
                        mybir.ActivationFunctionType.Copy,
                        scale=recipT[:, it:it + 1],
                    )
                else:
                    nc.vector.tensor_scalar_mul(ost[:], ps[:],
                                                recipT[:, it:it + 1])
                nc.sync.dma_start(
                    out[it * P:(it + 1) * P, fc * NCH + a:fc * NCH + b],
                    ost[:])


def _get_nc(repeats=1):
    key = ("nc", repeats)
    if key not in _CACHE:
        _CACHE[key] = _build(repeats)
    return _CACHE[key]


def _prep_inputs(x, Wq, Wk, Wv):
    f16 = np.float16
    x = np.asarray(x, dtype=np.float32)
    wq_t = np.ascontiguousarray(np.asarray(Wq, dtype=np.float32).T.astype(f16))
    wk_t = np.ascontiguousarray(np.asarray(Wk, dtype=np.float32).T.astype(f16))
    wv_t = np.ascontiguousarray(np.asarray(Wv, dtype=np.float32).T.astype(f16))
    xt_b = [np.ascontiguousarray(x[b].T.astype(f16)) for b in range(B)]
    in_maps = []
    for c in range(N_CORES):
        b, h = divmod(c, 2)
        xq_c = np.ascontiguousarray(x[b][h * SQ:(h + 1) * SQ].T.astype(f16))
        in_maps.append({"xq": xq_c, "xt": xt_b[b],
                        "wq": wq_t, "wk": wk_t, "wv": wv_t})
    return in_maps


def _get_runner():
    """Cached jitted dispatcher: one XLA/NEFF compile per process, reused
    across kernel() calls (run_bass_kernel_spmd would recompile per call)."""
    if "runner" in _CACHE:
        return _CACHE["runner"]
    import jax
    from jax.sharding import Mesh, PartitionSpec
    from jax.experimental.shard_map import shard_map
    from concourse.bass2jax import (
        _bass_exec_p, install_neuronx_cc_hook, partition_id_tensor)

    nc = _get_nc()
    install_neuronx_cc_hook()

    in_names, out_names, out_avals = [], [], []
    partition_name = nc.partition_id_tensor.name if nc.partition_id_tensor else None
    for alloc in nc.m.functions[0].allocations:
        if not isinstance(alloc, mybir.MemoryLocationSet):
            continue
        name = alloc.memorylocations[0].name
        if alloc.kind == "ExternalInput":
            if name != partition_name:
                in_names.append(name)
        elif alloc.kind == "ExternalOutput":
            out_names.append(name)
            out_avals.append(jax.core.ShapedArray(
                tuple(alloc.tensor_shape), mybir.dt.np(alloc.dtype)))
    n_params = len(in_names)
    all_names = list(in_names) + out_names
    if partition_name is not None:
        all_names.append(partition_name)

    def _body(*args):
        operands = list(args)
        if partition_name is not None:
            operands.append(partition_id_tensor())
        return tuple(_bass_exec_p.bind(
            *operands,
            out_avals=tuple(out_avals),
            in_names=tuple(all_names),
            out_names=tuple(out_names),
            lowering_input_output_aliases=(),
            sim_require_finite=True,
            sim_require_nnan=True,
            nc=nc,
        ))

    devices = jax.devices()[:N_CORES]
    mesh = Mesh(np.asarray(devices), ("core",))
    nspecs = (PartitionSpec("core"),) * (n_params + len(out_names))
    sharded = jax.jit(
        shard_map(_body, mesh=mesh, in_specs=nspecs,
                  out_specs=(PartitionSpec("core"),) * len(out_names),
                  check_rep=False),
        keep_unused=True,
    )

    def run(in_maps):
        concat_in = [
            np.concatenate([in_maps[c][name] for c in range(N_CORES)], axis=0)
            for name in in_names
        ]
        concat_zero = [
            np.zeros((N_CORES * a.shape[0], *a.shape[1:]), a.dtype)
            for a in out_avals
        ]
        outs = sharded(*concat_in, *concat_zero)
        return {
            name: np.asarray(outs[i]).reshape(N_CORES, *out_avals[i].shape)
            for i, name in enumerate(out_names)
        }

    _CACHE["runner"] = run
    return run


def kernel(x, Wq, Wk, Wv):
    in_maps = _prep_inputs(x, Wq, Wk, Wv)
    res = _get_runner()(in_maps)
    out = np.empty((B, S, D), dtype=np.float32)
    for c in range(N_CORES):
        b, h = divmod(c, 2)
        out[b, h * SQ:(h + 1) * SQ, :] = res["out"][c]
    return out


# revision 41
# speedup vs baseline: 1.0098x; 1.0005x over previous
"""Trainium2 Bass kernel: single-head self-attention.

Reference computation (fp32):
    q = x @ Wq.T ; k = x @ Wk.T ; v = x @ Wv.T        (x: [4, 2048, 1024])
    out = softmax((q @ k.T) / 32) @ v                 ([4, 2048, 1024])

Sharding: 8 cores = (batch 4) x (query halves 2). Each core owns 1024 query
rows of one batch element. k is recomputed for the full sequence on both
cores of a pair (computing the remote k half locally costs ~27us of TensorE
and replaces an AllGather that would serialize behind the v exchange on the
collective cores for ~120us). v is computed for the own half only and
exchanged as TWO column-half pair-wise AllGathers: the first is dispatched
mid-way through phase V (~23us in), the second at the end of phase V, so
they pipeline on the collective cores and complete (~100us / ~168us) before
the PV pass that consumes each half (~168us / ~196us) -- the exchange is
fully hidden behind the k/q/scores matmul stream.

SPMD symmetry: the program must not depend on the core's rank, so the host
supplies both x[b].T in global order (xt, for the k projection) and the
core's own query columns (xq, for the q and v-own projections). The v
exchange works in global row order (each core's own rows land at their
global position in the AllGather output), which keeps the j-order of
scores and PV consistent across the pair without rank-dependent addressing.

All matmul operands are fp16 (same TensorE throughput as bf16 on TRN2 --
both upconvert to FP22 in the PE -- but 10 mantissa bits instead of 7:
measured rel-absmax error 4e-4 vs 3.9e-3 for bf16). Accumulation is fp32
in PSUM. Softmax max-subtraction is unnecessary (|scores/32| < ~2.6 by
construction), so ScalarE applies exp(scores/32) directly out of PSUM.
The per-query denominators accumulate into one [i-part, 8] PSUM bank via
N=1 matmuls against a ones column, emitted four jt behind the scores
stream (single accumulation group: one bank-clearing start, per-element
has_written handles per-column accumulation; the lag keeps them out of
the PE's 4-deep wait queue, which would otherwise block the sequencer on
the exp drains they read), so a single reciprocal yields all drain
scales and the PV tail has no denominator dependency.

Each tensor lives in ONE wide SBUF tile loaded by ONE 3D-access-pattern
DMA (descriptor generation serializes ~0.6us per DMA instruction on the
shared HWDGE, so instruction count is what matters, not transfer split).
The w ring (bufs=2) carries wv -> wk -> wq -> va -> vb: each allocation's
DMA fires when the buffer two allocations back is released, which both
prefetches the next phase's operand under the current phase's matmuls and
lets the gathered v reuse the dead weight space.

Phase order is V, Q, K, S, PV: Q runs before K so its trailing qt
drains (which every S chunk waits on via engine-progress semaphores)
complete under K's 55us of matmuls, while K->S hands off pipelined
(each S chunk's kt column slices are written early in each ft's jc
sweep). The final PV group runs as two half-column groups so the first
half's drain+store pipeline under the second half's matmuls.

Per-core TensorE work: ~218us of N<=512 fp16 matmul streaming at 2.4 GHz
plus 128 N=1 denominator matmuls; drains run on ScalarE/VectorE under the
matmul stream; both AllGathers and all DMA overlap compute. Cost-model
(TimelineSim) total ~228us vs 372us for the previous two-AllGather bf16
variant: the matmul stream runs gap-free from ~12us (startup is
DMA-bandwidth-bound) to the end, within ~3% of its 218us floor.
"""

import numpy as np
from contextlib import ExitStack

import concourse.bacc as bacc
import concourse.tile as tile
import concourse.mybir as mybir

F16 = mybir.dt.float16
F32 = mybir.dt.float32
P = 128
B, S, D = 4, 2048, 1024
SQ = S // 2   # query rows per core
N_CORES = 8
ET = D // P   # contraction tiles over embed dim (projections)
FT = D // P   # feature tiles
JT = S // P   # kv-sequence tiles
IT = SQ // P  # query tiles
NCH = 512     # moving-operand chunk (one fp32 PSUM bank)
INV_SQRT_D = 1.0 / 32.0

_CACHE: dict = {}


def _g3(dram_ap, cols=None):
    """[G*128, C] DRAM slice -> [128, G, C] access pattern (rows = g*128+p)."""
    if cols is not None:
        dram_ap = dram_ap[:, cols[0]:cols[1]]
    return dram_ap.rearrange("(g p) c -> p g c", p=P)


def _s3(tile_ap, width, cols=None):
    """[128, G*width] SBUF tile view -> [128, G, C] matching _g3."""
    v = tile_ap[:].rearrange("p (g c) -> p g c", c=width)
    if cols is not None:
        v = v[:, :, cols[0]:cols[1]]
    return v


def _build(repeats=1):
    nc = bacc.Bacc("TRN2", target_bir_lowering=False, debug=False, num_devices=N_CORES)
    xq = nc.dram_tensor("xq", [D, SQ], F16, kind="ExternalInput").ap()
    xt = nc.dram_tensor("xt", [D, S], F16, kind="ExternalInput").ap()
    wq = nc.dram_tensor("wq", [D, D], F16, kind="ExternalInput").ap()
    wk = nc.dram_tensor("wk", [D, D], F16, kind="ExternalInput").ap()
    wv = nc.dram_tensor("wv", [D, D], F16, kind="ExternalInput").ap()
    out = nc.dram_tensor("out", [SQ, D], F32, kind="ExternalOutput").ap()

    with tile.TileContext(nc) as tc, ExitStack() as ctx:
        x_pool = ctx.enter_context(tc.tile_pool(name="x", bufs=1))
        w_pool = ctx.enter_context(tc.tile_pool(name="w", bufs=1))
        qt_pool = ctx.enter_context(tc.tile_pool(name="qt", bufs=1))
        kt_pool = ctx.enter_context(tc.tile_pool(name="kt", bufs=1))
        exp_pool = ctx.enter_context(tc.tile_pool(name="expT", bufs=1))
        stage_pool = ctx.enter_context(tc.tile_pool(name="stage", bufs=1))
        small_pool = ctx.enter_context(tc.tile_pool(name="small", bufs=1))
        mm_psum = ctx.enter_context(tc.tile_pool(name="mmps", bufs=7, space="PSUM"))
        dn_psum = ctx.enter_context(tc.tile_pool(name="dnps", bufs=1, space="PSUM"))
        dram_pool = ctx.enter_context(tc.tile_pool(name="dram", bufs=1, space="DRAM"))

        xq_t = x_pool.tile([P, ET * SQ], F16, name="xq_t")
        xt_t = x_pool.tile([P, ET * S], F16, name="xt_t")

        def ring(name):
            return w_pool.tile([P, ET * D], F16, name=name, tag="wring", bufs=2)

        wv_t = ring("wv_t")
        wq_t = ring("wq_t")
        # DMA issue order is the service order on the shared DMA engines:
        # front-load what phase V's first chains need (xq j-slice 0 + the
        # first wv column half), then the rest lands under compute.
        nc.sync.dma_start(_s3(xq_t, SQ, (0, 2 * P)), _g3(xq, (0, 2 * P)))
        nc.sync.dma_start(_s3(wv_t, D, (0, NCH // 2)), _g3(wv, (0, NCH // 2)))
        nc.sync.dma_start(_s3(xq_t, SQ, (2 * P, NCH)), _g3(xq, (2 * P, NCH)))
        nc.sync.dma_start(_s3(wv_t, D, (NCH // 2, NCH)), _g3(wv, (NCH // 2, NCH)))
        nc.sync.dma_start(_s3(xq_t, SQ, (NCH, SQ)), _g3(xq, (NCH, SQ)))
        nc.sync.dma_start(_s3(wv_t, D, (NCH, D)), _g3(wv, (NCH, D)))
        nc.sync.dma_start(_s3(wq_t, D), _g3(wq))
        nc.sync.dma_start(_s3(xt_t, S, (0, NCH)), _g3(xt, (0, NCH)))
        nc.sync.dma_start(_s3(xt_t, S, (NCH, 2 * NCH)), _g3(xt, (NCH, 2 * NCH)))
        nc.sync.dma_start(_s3(xt_t, S, (2 * NCH, S)), _g3(xt, (2 * NCH, S)))

        for _rep in range(repeats):
            _compute(nc, tc, xq_t, xt_t, wv_t, wq_t, ring, wk, out,
                     qt_pool, kt_pool, exp_pool, stage_pool, small_pool,
                     mm_psum, dn_psum, dram_pool)

    nc.compile()
    return nc


def _compute(nc, tc, xq_t, xt_t, wv_t, wq_t, ring, wk, out,
             qt_pool, kt_pool, exp_pool, stage_pool, small_pool,
             mm_psum, dn_psum, dram_pool):
    groups = [[0, 1], [2, 3], [4, 5], [6, 7]]
    # v is exchanged in two column-half AllGathers: the fc0 half is staged
    # and dispatched mid-way through phase V, so the two collectives
    # pipeline on the collective cores and both complete long before the
    # PV pass that consumes them.
    kv_in = [dram_pool.tile([SQ, NCH], F16, name=f"kv_in{fc}")
             for fc in range(D // NCH)]
    kv_out = [dram_pool.tile([S, NCH], F16, name=f"kv_out{fc}")
              for fc in range(D // NCH)]

    def xqs(et, a, b):
        return xq_t[:, et * SQ + a:et * SQ + b]

    def xts(et, a, b):
        return xt_t[:, et * S + a:et * S + b]

    def ws(w_t, et, a, b):
        return w_t[:, et * D + a:et * D + b]

    # A short burst of throwaway matmuls while the first input slabs are
    # still in flight: costs nothing (PE would be idle) and pays the PE
    # p-state/HAM warm-up ramp before the real stream begins.
    # The warm-up matmuls borrow the denominator pool's bank: the later
    # denominator group opens with start=True, which clears the bank, so
    # the junk it leaves behind is harmless and mm_psum keeps 7 banks.
    warm = small_pool.tile([P, NCH], F16, name="warm")
    nc.vector.memset(warm[:], 0.0)
    psw = dn_psum.tile([P, NCH], F32, name="ps_w", tag="dn")
    for _ in range(7):
        nc.tensor.matmul(psw[:], warm[:, 0:P], warm[:], start=True, stop=True)

    # ---- Phase V: v-own[j_own, f] = x_own @ Wv.T, staged to DRAM for the
    # AllGather. Own rows land at their global position on both cores, so
    # kv_out is in global j-order. fc-outer so the first pass only needs
    # the first wv column half.
    vstage = [stage_pool.tile([P, IT * NCH], F16, name=f"vstage{fc}")
              for fc in range(D // NCH)]
    # The first three jq chunks run as half-width column sub-chunks, all
    # first-quarter groups before any second-quarter group (one accumulation
    # group per bank: single bank-clearing start, per-element has_written
    # makes the second sub-chunk's first write an overwrite). The PE can
    # then stream three chunks off the first wv quarter + first xq slab
    # while the rest of the input is still in flight.
    H = NCH // 2
    NEARLY = 4
    early = [mm_psum.tile([P, NCH], F32, name="ps_v", tag="mm")
             for _ in range(NEARLY)]
    for sub in range(2):
        for jq in range(NEARLY):
            for et in range(ET):
                nc.tensor.matmul(
                    early[jq][:, sub * H:(sub + 1) * H],
                    xqs(et, jq * P, (jq + 1) * P),
                    ws(wv_t, et, sub * H, (sub + 1) * H),
                    start=(sub == 0 and et == 0),
                    stop=(sub == 1 and et == ET - 1),
                )
    for jq in range(NEARLY):
        nc.scalar.activation(
            vstage[0][:, jq * NCH:(jq + 1) * NCH], early[jq][:],
            mybir.ActivationFunctionType.Copy)
    for fc in range(D // NCH):
        for jq in range(NEARLY if fc == 0 else 0, SQ // P):
            ps = mm_psum.tile([P, NCH], F32, name="ps_v", tag="mm")
            for et in range(ET):
                nc.tensor.matmul(
                    ps[:],
                    xqs(et, jq * P, (jq + 1) * P),
                    ws(wv_t, et, fc * NCH, (fc + 1) * NCH),
                    start=(et == 0),
                    stop=(et == ET - 1),
                )
            nc.scalar.activation(
                vstage[fc][:, jq * NCH:(jq + 1) * NCH], ps[:],
                mybir.ActivationFunctionType.Copy)
        nc.sync.dma_start(_g3(kv_in[fc].opt()), _s3(vstage[fc], NCH))
        nc.gpsimd.collective_compute(
            "AllGather", mybir.AluOpType.bypass, replica_groups=groups,
            ins=[kv_in[fc].opt()], outs=[kv_out[fc].opt()],
        )

    wk_t = ring("wk_t")  # ring slot frees at end of phase V; loads during Q
    nc.sync.dma_start(_s3(wk_t, D), _g3(wk))

    # ---- Phase Q: qT[f, i] for the own query half. Q runs BEFORE K so its
    # trailing drains (which phase S waits on via engine-progress
    # semaphores, since every S chunk reads a full column half of qt)
    # complete under K's 55us of matmuls instead of right at S's entry.
    qt_t = qt_pool.tile([P, FT * SQ], F16, name="qt_t")
    for ic in range(SQ // NCH):
        for ft in range(FT):
            ps = mm_psum.tile([P, NCH], F32, name="ps_q", tag="mm")
            for et in range(ET):
                nc.tensor.matmul(
                    ps[:],
                    ws(wq_t, et, ft * P, (ft + 1) * P),
                    xqs(et, ic * NCH, (ic + 1) * NCH),
                    start=(et == 0),
                    stop=(et == ET - 1),
                )
            qdst = qt_t[:, ft * SQ + ic * NCH:ft * SQ + (ic + 1) * NCH]
            if ft % 2 == 0:
                nc.vector.tensor_copy(qdst, ps[:])
            else:
                nc.scalar.activation(qdst, ps[:],
                                     mybir.ActivationFunctionType.Copy)

    # v reuses the ring: va evicts wq (released end of Q), vb evicts wk
    # (released end of K); the DMAs additionally wait on their AllGather's
    # output.
    va_t = ring("va_t")
    nc.sync.dma_start(_s3(va_t, NCH), _g3(kv_out[0]))

    # ---- Phase K: kT[f, j] = (x @ Wk.T).T for the FULL sequence (recomputed
    # locally instead of a second, serialized AllGather). K feeds S with an
    # intrinsically pipelined handoff: each S chunk's kt column slices are
    # written early within each ft's jc sweep.
    kt_t = kt_pool.tile([P, FT * S], F16, name="kt_t")
    for ft in range(FT):
        for jc in range(S // NCH):
            ps = mm_psum.tile([P, NCH], F32, name="ps_k", tag="mm")
            for et in range(ET):
                nc.tensor.matmul(
                    ps[:],
                    ws(wk_t, et, ft * P, (ft + 1) * P),
                    xts(et, jc * NCH, (jc + 1) * NCH),
                    start=(et == 0),
                    stop=(et == ET - 1),
                )
            dst = kt_t[:, ft * S + jc * NCH:ft * S + (jc + 1) * NCH]
            if jc % 2 == 0:
                nc.vector.tensor_copy(dst, ps[:])
            else:
                nc.scalar.activation(dst, ps[:],
                                     mybir.ActivationFunctionType.Copy)

    vb_t = ring("vb_t")
    nc.sync.dma_start(_s3(vb_t, NCH), _g3(kv_out[1]))

    def v_sl(fc, jt):
        t = va_t if fc == 0 else vb_t
        return t[:, jt * NCH:(jt + 1) * NCH]

    # ---- Phase S: expT[j, i] = exp(kT.T @ qT / 32), with the softmax
    # denominators accumulating into one [i-part, 8] PSUM bank via N=1
    # matmuls, one jt behind the scores stream.
    ones_t = small_pool.tile([P, 16], F16, name="ones")
    nc.vector.memset(ones_t[:], 1.0)
    ones_f16 = ones_t[:, 0:1]
    recipT = small_pool.tile([P, IT], F32, name="recipT")
    psd = dn_psum.tile([P, IT], F32, name="ps_d", tag="dn")
    exp_t = exp_pool.tile([P, JT * SQ], F16, name="exp_t")

    def exp_sl(jt, a, b):
        return exp_t[:, jt * SQ + a:jt * SQ + b]

    def emit_denoms(jt):
        for it in range(IT):
            nc.tensor.matmul(
                psd[:, it:it + 1],
                exp_sl(jt, it * P, (it + 1) * P),
                ones_f16,
                start=(jt == 0 and it == 0),
                stop=(jt == JT - 1 and it == IT - 1),
            )

    for jt in range(JT):
        for ic in range(SQ // NCH):
            ps = mm_psum.tile([P, NCH], F32, name="ps_s", tag="mm")
            for ft in range(FT):
                nc.tensor.matmul(
                    ps[:],
                    kt_t[:, ft * S + jt * P:ft * S + (jt + 1) * P],
                    qt_t[:, ft * SQ + ic * NCH:ft * SQ + (ic + 1) * NCH],
                    start=(ft == 0),
                    stop=(ft == FT - 1),
                )
            nc.scalar.activation(
                exp_sl(jt, ic * NCH, (ic + 1) * NCH),
                ps[:],
                mybir.ActivationFunctionType.Exp,
                scale=INV_SQRT_D,
            )
        # lag the denominator matmuls 4 jt behind the scores stream: they
        # read both ic-halves of exp[jt'], and if the second half's drain is
        # still in flight they occupy the PE's 4-deep wait queue and block
        # the sequencer (measured as 2x ~1.1us stalls at lag 2).
        if jt > 3:
            emit_denoms(jt - 4)
    for jtl in range(JT - 4, JT - 1):
        emit_denoms(jtl)

    # ---- Phase PV: out[i, f] = (expT.T @ v) * recip[i], normalization
    # folded into the drain as a per-partition scale. Two passes, one per
    # v column half, so pass A only needs the first AllGather's output.
    # The last jt's denominators (whose exp drain is still in flight at
    # the end of phase S) slot in behind the first matmul group; the
    # reciprocal only gates the first drain, not the matmul stream.
    H = NCH // 2
    for fc in range(D // NCH):
        for it in range(IT):
            last_group = fc == D // NCH - 1 and it == IT - 1
            # The final group runs as two half-column groups in separate
            # banks so the first half's drain + store pipeline under the
            # second half's matmuls, shortening the exposed tail chain.
            subs = ((0, H), (H, NCH)) if last_group else ((0, NCH),)
            for a, b in subs:
                ps = mm_psum.tile([P, b - a], F32, name=f"ps_o{fc}", tag="mm")
                for jt in range(JT):
                    nc.tensor.matmul(ps[:], exp_sl(jt, it * P, (it + 1) * P),
                                     v_sl(fc, jt)[:, a:b],
                                     start=(jt == 0), stop=(jt == JT - 1))
                if fc == 0 and it == 0:
                    emit_denoms(JT - 1)
                    nc.vector.reciprocal(recipT[:], psd[:])
                ost = stage_pool.tile([P, b - a], F32, name="ostage",
                                      tag="ost", bufs=4)
                if it % 2 == 0:
                    nc.scalar.activation(
                        ost[:],
                        ps[:],
                        mybir.ActivationFunctionType.Copy,
                        scale=recipT[:, it:it + 1],
                    )
                else:
                    nc.vector.tensor_scalar_mul(ost[:], ps[:],
                                                recipT[:, it:it + 1])
                nc.sync.dma_start(
                    out[it * P:(it + 1) * P, fc * NCH + a:fc * NCH + b],
                    ost[:])


def _get_nc(repeats=1):
    key = ("nc", repeats)
    if key not in _CACHE:
        _CACHE[key] = _build(repeats)
    return _CACHE[key]


def _prep_inputs(x, Wq, Wk, Wv):
    f16 = np.float16
    x = np.asarray(x, dtype=np.float32)
    wq_t = np.ascontiguousarray(np.asarray(Wq, dtype=np.float32).T.astype(f16))
    wk_t = np.ascontiguousarray(np.asarray(Wk, dtype=np.float32).T.astype(f16))
    wv_t = np.ascontiguousarray(np.asarray(Wv, dtype=np.float32).T.astype(f16))
    xt_b = [np.ascontiguousarray(x[b].T.astype(f16)) for b in range(B)]
    in_maps = []
    for c in range(N_CORES):
        b, h = divmod(c, 2)
        xq_c = np.ascontiguousarray(x[b][h * SQ:(h + 1) * SQ].T.astype(f16))
        in_maps.append({"xq": xq_c, "xt": xt_b[b],
                        "wq": wq_t, "wk": wk_t, "wv": wv_t})
    return in_maps


def _get_runner():
    """Cached jitted dispatcher: one XLA/NEFF compile per process, reused
    across kernel() calls (run_bass_kernel_spmd would recompile per call)."""
    if "runner" in _CACHE:
        return _CACHE["runner"]
    import jax
    from jax.sharding import Mesh, PartitionSpec
    from jax.experimental.shard_map import shard_map
    from concourse.bass2jax import (
        _bass_exec_p, install_neuronx_cc_hook, partition_id_tensor)

    nc = _get_nc()
    install_neuronx_cc_hook()

    in_names, out_names, out_avals = [], [], []
    partition_name = nc.partition_id_tensor.name if nc.partition_id_tensor else None
    for alloc in nc.m.functions[0].allocations:
        if not isinstance(alloc, mybir.MemoryLocationSet):
            continue
        name = alloc.memorylocations[0].name
        if alloc.kind == "ExternalInput":
            if name != partition_name:
                in_names.append(name)
        elif alloc.kind == "ExternalOutput":
            out_names.append(name)
            out_avals.append(jax.core.ShapedArray(
                tuple(alloc.tensor_shape), mybir.dt.np(alloc.dtype)))
    n_params = len(in_names)
    all_names = list(in_names) + out_names
    if partition_name is not None:
        all_names.append(partition_name)

    def _body(*args):
        operands = list(args)
        if partition_name is not None:
            operands.append(partition_id_tensor())
        return tuple(_bass_exec_p.bind(
            *operands,
            out_avals=tuple(out_avals),
            in_names=tuple(all_names),
            out_names=tuple(out_names),
            lowering_input_output_aliases=(),
            sim_require_finite=True,
            sim_require_nnan=True,
            nc=nc,
        ))

    devices = jax.devices()[:N_CORES]
    mesh = Mesh(np.asarray(devices), ("core",))
    nspecs = (PartitionSpec("core"),) * (n_params + len(out_names))
    sharded = jax.jit(
        shard_map(_body, mesh=mesh, in_specs=nspecs,
                  out_specs=(PartitionSpec("core"),) * len(out_names),
                  check_rep=False),
        keep_unused=True,
    )

    def run(in_maps):
        concat_in = [
            np.concatenate([in_maps[c][name] for c in range(N_CORES)], axis=0)
            for name in in_names
        ]
        concat_zero = [
            np.zeros((N_CORES * a.shape[0], *a.shape[1:]), a.dtype)
            for a in out_avals
        ]
        outs = sharded(*concat_in, *concat_zero)
        return {
            name: np.asarray(outs[i]).reshape(N_CORES, *out_avals[i].shape)
            for i, name in enumerate(out_names)
        }

    _CACHE["runner"] = run
    return run


def kernel(x, Wq, Wk, Wv):
    in_maps = _prep_inputs(x, Wq, Wk, Wv)
    res = _get_runner()(in_maps)
    out = np.empty((B, S, D), dtype=np.float32)
    for c in range(N_CORES):
        b, h = divmod(c, 2)
        out[b, h * SQ:(h + 1) * SQ, :] = res["out"][c]
    return out


# revision 42
# speedup vs baseline: 1.0108x; 1.0010x over previous
"""Trainium2 Bass kernel: single-head self-attention.

Reference computation (fp32):
    q = x @ Wq.T ; k = x @ Wk.T ; v = x @ Wv.T        (x: [4, 2048, 1024])
    out = softmax((q @ k.T) / 32) @ v                 ([4, 2048, 1024])

Sharding: 8 cores = (batch 4) x (query halves 2). Each core owns 1024 query
rows of one batch element. k is recomputed for the full sequence on both
cores of a pair (computing the remote k half locally costs ~27us of TensorE
and replaces an AllGather that would serialize behind the v exchange on the
collective cores for ~120us). v is computed for the own half only and
exchanged as TWO column-half pair-wise AllGathers: the first is dispatched
mid-way through phase V (~23us in), the second at the end of phase V, so
they pipeline on the collective cores and complete (~100us / ~168us) before
the PV pass that consumes each half (~168us / ~196us) -- the exchange is
fully hidden behind the k/q/scores matmul stream.

SPMD symmetry: the program must not depend on the core's rank, so the host
supplies both x[b].T in global order (xt, for the k projection) and the
core's own query columns (xq, for the q and v-own projections). The v
exchange works in global row order (each core's own rows land at their
global position in the AllGather output), which keeps the j-order of
scores and PV consistent across the pair without rank-dependent addressing.

All matmul operands are fp16 (same TensorE throughput as bf16 on TRN2 --
both upconvert to FP22 in the PE -- but 10 mantissa bits instead of 7:
measured rel-absmax error 4e-4 vs 3.9e-3 for bf16). Accumulation is fp32
in PSUM. Softmax max-subtraction is unnecessary (|scores/32| < ~2.6 by
construction), so ScalarE applies exp(scores/32) directly out of PSUM.
The per-query denominators accumulate into one [i-part, 8] PSUM bank via
N=1 matmuls against a ones column, emitted four jt behind the scores
stream (single accumulation group: one bank-clearing start, per-element
has_written handles per-column accumulation; the lag keeps them out of
the PE's 4-deep wait queue, which would otherwise block the sequencer on
the exp drains they read), so a single reciprocal yields all drain
scales and the PV tail has no denominator dependency.

Each tensor lives in ONE wide SBUF tile loaded by ONE 3D-access-pattern
DMA (descriptor generation serializes ~0.6us per DMA instruction on the
shared HWDGE, so instruction count is what matters, not transfer split).
The w ring (bufs=2) carries wv -> wk -> wq -> va -> vb: each allocation's
DMA fires when the buffer two allocations back is released, which both
prefetches the next phase's operand under the current phase's matmuls and
lets the gathered v reuse the dead weight space.

Phase order is V, Q, K, S, PV: Q runs before K so its trailing qt
drains (which every S chunk waits on via engine-progress semaphores)
complete under K's 55us of matmuls, while K->S hands off pipelined
(each S chunk's kt column slices are written early in each ft's jc
sweep). The final PV group runs as two half-column groups so the first
half's drain+store pipeline under the second half's matmuls.

Per-core TensorE work: ~218us of N<=512 fp16 matmul streaming at 2.4 GHz
plus 128 N=1 denominator matmuls; drains run on ScalarE/VectorE under the
matmul stream; both AllGathers and all DMA overlap compute. Cost-model
(TimelineSim) total ~228us vs 372us for the previous two-AllGather bf16
variant: the matmul stream runs gap-free from ~12us (startup is
DMA-bandwidth-bound) to the end, within ~3% of its 218us floor.
"""

import numpy as np
from contextlib import ExitStack

import concourse.bacc as bacc
import concourse.tile as tile
import concourse.mybir as mybir

F16 = mybir.dt.float16
F32 = mybir.dt.float32
P = 128
B, S, D = 4, 2048, 1024
SQ = S // 2   # query rows per core
N_CORES = 8
ET = D // P   # contraction tiles over embed dim (projections)
FT = D // P   # feature tiles
JT = S // P   # kv-sequence tiles
IT = SQ // P  # query tiles
NCH = 512     # moving-operand chunk (one fp32 PSUM bank)
INV_SQRT_D = 1.0 / 32.0

_CACHE: dict = {}


def _g3(dram_ap, cols=None):
    """[G*128, C] DRAM slice -> [128, G, C] access pattern (rows = g*128+p)."""
    if cols is not None:
        dram_ap = dram_ap[:, cols[0]:cols[1]]
    return dram_ap.rearrange("(g p) c -> p g c", p=P)


def _s3(tile_ap, width, cols=None):
    """[128, G*width] SBUF tile view -> [128, G, C] matching _g3."""
    v = tile_ap[:].rearrange("p (g c) -> p g c", c=width)
    if cols is not None:
        v = v[:, :, cols[0]:cols[1]]
    return v


def _build(repeats=1):
    nc = bacc.Bacc("TRN2", target_bir_lowering=False, debug=False, num_devices=N_CORES)
    xq = nc.dram_tensor("xq", [D, SQ], F16, kind="ExternalInput").ap()
    xt = nc.dram_tensor("xt", [D, S], F16, kind="ExternalInput").ap()
    wq = nc.dram_tensor("wq", [D, D], F16, kind="ExternalInput").ap()
    wk = nc.dram_tensor("wk", [D, D], F16, kind="ExternalInput").ap()
    wv = nc.dram_tensor("wv", [D, D], F16, kind="ExternalInput").ap()
    out = nc.dram_tensor("out", [SQ, D], F32, kind="ExternalOutput").ap()

    with tile.TileContext(nc) as tc, ExitStack() as ctx:
        x_pool = ctx.enter_context(tc.tile_pool(name="x", bufs=1))
        w_pool = ctx.enter_context(tc.tile_pool(name="w", bufs=1))
        qt_pool = ctx.enter_context(tc.tile_pool(name="qt", bufs=1))
        kt_pool = ctx.enter_context(tc.tile_pool(name="kt", bufs=1))
        exp_pool = ctx.enter_context(tc.tile_pool(name="expT", bufs=1))
        stage_pool = ctx.enter_context(tc.tile_pool(name="stage", bufs=1))
        small_pool = ctx.enter_context(tc.tile_pool(name="small", bufs=1))
        mm_psum = ctx.enter_context(tc.tile_pool(name="mmps", bufs=7, space="PSUM"))
        dn_psum = ctx.enter_context(tc.tile_pool(name="dnps", bufs=1, space="PSUM"))
        dram_pool = ctx.enter_context(tc.tile_pool(name="dram", bufs=1, space="DRAM"))

        xq_t = x_pool.tile([P, ET * SQ], F16, name="xq_t")
        xt_t = x_pool.tile([P, ET * S], F16, name="xt_t")

        def ring(name):
            return w_pool.tile([P, ET * D], F16, name=name, tag="wring", bufs=2)

        wv_t = ring("wv_t")
        wq_t = ring("wq_t")
        # DMA issue order is the service order on the shared DMA engines:
        # front-load what phase V's first chains need (xq j-slice 0 + the
        # first wv column half), then the rest lands under compute.
        nc.sync.dma_start(_s3(xq_t, SQ, (0, 2 * P)), _g3(xq, (0, 2 * P)))
        nc.sync.dma_start(_s3(wv_t, D, (0, NCH // 2)), _g3(wv, (0, NCH // 2)))
        nc.sync.dma_start(_s3(xq_t, SQ, (2 * P, NCH)), _g3(xq, (2 * P, NCH)))
        nc.sync.dma_start(_s3(wv_t, D, (NCH // 2, NCH)), _g3(wv, (NCH // 2, NCH)))
        nc.sync.dma_start(_s3(xq_t, SQ, (NCH, SQ)), _g3(xq, (NCH, SQ)))
        nc.sync.dma_start(_s3(wv_t, D, (NCH, D)), _g3(wv, (NCH, D)))
        nc.sync.dma_start(_s3(wq_t, D), _g3(wq))
        nc.sync.dma_start(_s3(xt_t, S, (0, NCH)), _g3(xt, (0, NCH)))
        nc.sync.dma_start(_s3(xt_t, S, (NCH, 2 * NCH)), _g3(xt, (NCH, 2 * NCH)))
        nc.sync.dma_start(_s3(xt_t, S, (2 * NCH, S)), _g3(xt, (2 * NCH, S)))

        for _rep in range(repeats):
            _compute(nc, tc, xq_t, xt_t, wv_t, wq_t, ring, wk, out,
                     qt_pool, kt_pool, exp_pool, stage_pool, small_pool,
                     mm_psum, dn_psum, dram_pool)

    nc.compile()
    return nc


def _compute(nc, tc, xq_t, xt_t, wv_t, wq_t, ring, wk, out,
             qt_pool, kt_pool, exp_pool, stage_pool, small_pool,
             mm_psum, dn_psum, dram_pool):
    groups = [[0, 1], [2, 3], [4, 5], [6, 7]]
    # v is exchanged in two column-half AllGathers: the fc0 half is staged
    # and dispatched mid-way through phase V, so the two collectives
    # pipeline on the collective cores and both complete long before the
    # PV pass that consumes them.
    kv_in = [dram_pool.tile([SQ, NCH], F16, name=f"kv_in{fc}")
             for fc in range(D // NCH)]
    kv_out = [dram_pool.tile([S, NCH], F16, name=f"kv_out{fc}")
              for fc in range(D // NCH)]

    def xqs(et, a, b):
        return xq_t[:, et * SQ + a:et * SQ + b]

    def xts(et, a, b):
        return xt_t[:, et * S + a:et * S + b]

    def ws(w_t, et, a, b):
        return w_t[:, et * D + a:et * D + b]

    # A short burst of throwaway matmuls while the first input slabs are
    # still in flight: costs nothing (PE would be idle) and pays the PE
    # p-state/HAM warm-up ramp before the real stream begins.
    # The warm-up matmuls borrow the denominator pool's bank: the later
    # denominator group opens with start=True, which clears the bank, so
    # the junk it leaves behind is harmless and mm_psum keeps 7 banks.
    warm = small_pool.tile([P, NCH], F16, name="warm")
    nc.vector.memset(warm[:], 0.0)
    psw = dn_psum.tile([P, NCH], F32, name="ps_w", tag="dn")
    for _ in range(7):
        nc.tensor.matmul(psw[:], warm[:, 0:P], warm[:], start=True, stop=True)

    # ---- Phase V: v-own[j_own, f] = x_own @ Wv.T, staged to DRAM for the
    # AllGather. Own rows land at their global position on both cores, so
    # kv_out is in global j-order. fc-outer so the first pass only needs
    # the first wv column half.
    vstage = [stage_pool.tile([P, IT * NCH], F16, name=f"vstage{fc}")
              for fc in range(D // NCH)]
    # The first three jq chunks run as half-width column sub-chunks, all
    # first-quarter groups before any second-quarter group (one accumulation
    # group per bank: single bank-clearing start, per-element has_written
    # makes the second sub-chunk's first write an overwrite). The PE can
    # then stream three chunks off the first wv quarter + first xq slab
    # while the rest of the input is still in flight.
    H = NCH // 2
    NEARLY = 4
    early = [mm_psum.tile([P, NCH], F32, name="ps_v", tag="mm")
             for _ in range(NEARLY)]
    for sub in range(2):
        for jq in range(NEARLY):
            for et in range(ET):
                nc.tensor.matmul(
                    early[jq][:, sub * H:(sub + 1) * H],
                    xqs(et, jq * P, (jq + 1) * P),
                    ws(wv_t, et, sub * H, (sub + 1) * H),
                    start=(sub == 0 and et == 0),
                    stop=(sub == 1 and et == ET - 1),
                )
    for jq in range(NEARLY):
        nc.scalar.activation(
            vstage[0][:, jq * NCH:(jq + 1) * NCH], early[jq][:],
            mybir.ActivationFunctionType.Copy)
    for fc in range(D // NCH):
        for jq in range(NEARLY if fc == 0 else 0, SQ // P):
            ps = mm_psum.tile([P, NCH], F32, name="ps_v", tag="mm")
            for et in range(ET):
                nc.tensor.matmul(
                    ps[:],
                    xqs(et, jq * P, (jq + 1) * P),
                    ws(wv_t, et, fc * NCH, (fc + 1) * NCH),
                    start=(et == 0),
                    stop=(et == ET - 1),
                )
            nc.scalar.activation(
                vstage[fc][:, jq * NCH:(jq + 1) * NCH], ps[:],
                mybir.ActivationFunctionType.Copy)
        nc.sync.dma_start(_g3(kv_in[fc].opt()), _s3(vstage[fc], NCH))
        nc.gpsimd.collective_compute(
            "AllGather", mybir.AluOpType.bypass, replica_groups=groups,
            ins=[kv_in[fc].opt()], outs=[kv_out[fc].opt()],
        )

    wk_t = ring("wk_t")  # ring slot frees at end of phase V; loads during Q
    nc.sync.dma_start(_s3(wk_t, D), _g3(wk))

    # ---- Phase Q: qT[f, i] for the own query half. Q runs BEFORE K so its
    # trailing drains (which phase S waits on via engine-progress
    # semaphores, since every S chunk reads a full column half of qt)
    # complete under K's 55us of matmuls instead of right at S's entry.
    qt_t = qt_pool.tile([P, FT * SQ], F16, name="qt_t")
    for ic in range(SQ // NCH):
        for ft in range(FT):
            ps = mm_psum.tile([P, NCH], F32, name="ps_q", tag="mm")
            for et in range(ET):
                nc.tensor.matmul(
                    ps[:],
                    ws(wq_t, et, ft * P, (ft + 1) * P),
                    xqs(et, ic * NCH, (ic + 1) * NCH),
                    start=(et == 0),
                    stop=(et == ET - 1),
                )
            qdst = qt_t[:, ft * SQ + ic * NCH:ft * SQ + (ic + 1) * NCH]
            if ft % 2 == 0:
                nc.vector.tensor_copy(qdst, ps[:])
            else:
                nc.scalar.activation(qdst, ps[:],
                                     mybir.ActivationFunctionType.Copy)

    # v reuses the ring: va evicts wq (released end of Q), vb evicts wk
    # (released end of K); the DMAs additionally wait on their AllGather's
    # output.
    va_t = ring("va_t")
    nc.sync.dma_start(_s3(va_t, NCH), _g3(kv_out[0]))

    # ---- Phase K: kT[f, j] = (x @ Wk.T).T for the FULL sequence (recomputed
    # locally instead of a second, serialized AllGather). K feeds S with an
    # intrinsically pipelined handoff: each S chunk's kt column slices are
    # written early within each ft's jc sweep.
    kt_t = kt_pool.tile([P, FT * S], F16, name="kt_t")
    for ft in range(FT):
        for jc in range(S // NCH):
            ps = mm_psum.tile([P, NCH], F32, name="ps_k", tag="mm")
            for et in range(ET):
                nc.tensor.matmul(
                    ps[:],
                    ws(wk_t, et, ft * P, (ft + 1) * P),
                    xts(et, jc * NCH, (jc + 1) * NCH),
                    start=(et == 0),
                    stop=(et == ET - 1),
                )
            dst = kt_t[:, ft * S + jc * NCH:ft * S + (jc + 1) * NCH]
            if jc % 2 == 0:
                nc.vector.tensor_copy(dst, ps[:])
            else:
                nc.scalar.activation(dst, ps[:],
                                     mybir.ActivationFunctionType.Copy)

    vb_t = ring("vb_t")
    nc.sync.dma_start(_s3(vb_t, NCH), _g3(kv_out[1]))

    def v_sl(fc, jt):
        t = va_t if fc == 0 else vb_t
        return t[:, jt * NCH:(jt + 1) * NCH]

    # ---- Phase S: expT[j, i] = exp(kT.T @ qT / 32), with the softmax
    # denominators accumulating into one [i-part, 8] PSUM bank via N=1
    # matmuls, one jt behind the scores stream.
    ones_t = small_pool.tile([P, 16], F16, name="ones")
    nc.vector.memset(ones_t[:], 1.0)
    ones_f16 = ones_t[:, 0:1]
    recipT = small_pool.tile([P, IT], F32, name="recipT")
    psd = dn_psum.tile([P, IT], F32, name="ps_d", tag="dn")
    exp_t = exp_pool.tile([P, JT * SQ], F16, name="exp_t")

    def exp_sl(jt, a, b):
        return exp_t[:, jt * SQ + a:jt * SQ + b]

    def emit_denoms(jt):
        for it in range(IT):
            nc.tensor.matmul(
                psd[:, it:it + 1],
                exp_sl(jt, it * P, (it + 1) * P),
                ones_f16,
                start=(jt == 0 and it == 0),
                stop=(jt == JT - 1 and it == IT - 1),
            )

    for jt in range(JT):
        for ic in range(SQ // NCH):
            ps = mm_psum.tile([P, NCH], F32, name="ps_s", tag="mm")
            for ft in range(FT):
                nc.tensor.matmul(
                    ps[:],
                    kt_t[:, ft * S + jt * P:ft * S + (jt + 1) * P],
                    qt_t[:, ft * SQ + ic * NCH:ft * SQ + (ic + 1) * NCH],
                    start=(ft == 0),
                    stop=(ft == FT - 1),
                )
            nc.scalar.activation(
                exp_sl(jt, ic * NCH, (ic + 1) * NCH),
                ps[:],
                mybir.ActivationFunctionType.Exp,
                scale=INV_SQRT_D,
            )
        # lag the denominator matmuls 4 jt behind the scores stream: they
        # read both ic-halves of exp[jt'], and if the second half's drain is
        # still in flight they occupy the PE's 4-deep wait queue and block
        # the sequencer (measured as 2x ~1.1us stalls at lag 2).
        if jt > 3:
            emit_denoms(jt - 4)
    for jtl in range(JT - 4, JT - 1):
        emit_denoms(jtl)

    # ---- Phase PV: out[i, f] = (expT.T @ v) * recip[i], normalization
    # folded into the drain as a per-partition scale. Two passes, one per
    # v column half, so pass A only needs the first AllGather's output.
    # The last jt's denominators (whose exp drain is still in flight at
    # the end of phase S) slot in behind the first matmul group; the
    # reciprocal only gates the first drain, not the matmul stream.
    H = NCH // 2
    for fc in range(D // NCH):
        for it in range(IT):
            last_group = fc == D // NCH - 1 and it == IT - 1
            # The final group runs as two half-column groups in separate
            # banks so the first half's drain + store pipeline under the
            # second half's matmuls, shortening the exposed tail chain.
            subs = ((0, H), (H, 3 * NCH // 4), (3 * NCH // 4, NCH)) \
                if last_group else ((0, NCH),)
            for a, b in subs:
                ps = mm_psum.tile([P, b - a], F32, name=f"ps_o{fc}", tag="mm")
                for jt in range(JT):
                    nc.tensor.matmul(ps[:], exp_sl(jt, it * P, (it + 1) * P),
                                     v_sl(fc, jt)[:, a:b],
                                     start=(jt == 0), stop=(jt == JT - 1))
                if fc == 0 and it == 0:
                    emit_denoms(JT - 1)
                    nc.vector.reciprocal(recipT[:], psd[:])
                ost = stage_pool.tile([P, b - a], F32, name="ostage",
                                      tag="ost", bufs=4)
                if it % 2 == 0:
                    nc.scalar.activation(
                        ost[:],
                        ps[:],
                        mybir.ActivationFunctionType.Copy,
                        scale=recipT[:, it:it + 1],
                    )
                else:
                    nc.vector.tensor_scalar_mul(ost[:], ps[:],
                                                recipT[:, it:it + 1])
                nc.sync.dma_start(
                    out[it * P:(it + 1) * P, fc * NCH + a:fc * NCH + b],
                    ost[:])


def _get_nc(repeats=1):
    key = ("nc", repeats)
    if key not in _CACHE:
        _CACHE[key] = _build(repeats)
    return _CACHE[key]


def _prep_inputs(x, Wq, Wk, Wv):
    f16 = np.float16
    x = np.asarray(x, dtype=np.float32)
    wq_t = np.ascontiguousarray(np.asarray(Wq, dtype=np.float32).T.astype(f16))
    wk_t = np.ascontiguousarray(np.asarray(Wk, dtype=np.float32).T.astype(f16))
    wv_t = np.ascontiguousarray(np.asarray(Wv, dtype=np.float32).T.astype(f16))
    xt_b = [np.ascontiguousarray(x[b].T.astype(f16)) for b in range(B)]
    in_maps = []
    for c in range(N_CORES):
        b, h = divmod(c, 2)
        xq_c = np.ascontiguousarray(x[b][h * SQ:(h + 1) * SQ].T.astype(f16))
        in_maps.append({"xq": xq_c, "xt": xt_b[b],
                        "wq": wq_t, "wk": wk_t, "wv": wv_t})
    return in_maps


def _get_runner():
    """Cached jitted dispatcher: one XLA/NEFF compile per process, reused
    across kernel() calls (run_bass_kernel_spmd would recompile per call)."""
    if "runner" in _CACHE:
        return _CACHE["runner"]
    import jax
    from jax.sharding import Mesh, PartitionSpec
    from jax.experimental.shard_map import shard_map
    from concourse.bass2jax import (
        _bass_exec_p, install_neuronx_cc_hook, partition_id_tensor)

    nc = _get_nc()
    install_neuronx_cc_hook()

    in_names, out_names, out_avals = [], [], []
    partition_name = nc.partition_id_tensor.name if nc.partition_id_tensor else None
    for alloc in nc.m.functions[0].allocations:
        if not isinstance(alloc, mybir.MemoryLocationSet):
            continue
        name = alloc.memorylocations[0].name
        if alloc.kind == "ExternalInput":
            if name != partition_name:
                in_names.append(name)
        elif alloc.kind == "ExternalOutput":
            out_names.append(name)
            out_avals.append(jax.core.ShapedArray(
                tuple(alloc.tensor_shape), mybir.dt.np(alloc.dtype)))
    n_params = len(in_names)
    all_names = list(in_names) + out_names
    if partition_name is not None:
        all_names.append(partition_name)

    def _body(*args):
        operands = list(args)
        if partition_name is not None:
            operands.append(partition_id_tensor())
        return tuple(_bass_exec_p.bind(
            *operands,
            out_avals=tuple(out_avals),
            in_names=tuple(all_names),
            out_names=tuple(out_names),
            lowering_input_output_aliases=(),
            sim_require_finite=True,
            sim_require_nnan=True,
            nc=nc,
        ))

    devices = jax.devices()[:N_CORES]
    mesh = Mesh(np.asarray(devices), ("core",))
    nspecs = (PartitionSpec("core"),) * (n_params + len(out_names))
    sharded = jax.jit(
        shard_map(_body, mesh=mesh, in_specs=nspecs,
                  out_specs=(PartitionSpec("core"),) * len(out_names),
                  check_rep=False),
        keep_unused=True,
    )

    def run(in_maps):
        concat_in = [
            np.concatenate([in_maps[c][name] for c in range(N_CORES)], axis=0)
            for name in in_names
        ]
        concat_zero = [
            np.zeros((N_CORES * a.shape[0], *a.shape[1:]), a.dtype)
            for a in out_avals
        ]
        outs = sharded(*concat_in, *concat_zero)
        return {
            name: np.asarray(outs[i]).reshape(N_CORES, *out_avals[i].shape)
            for i, name in enumerate(out_names)
        }

    _CACHE["runner"] = run
    return run


def kernel(x, Wq, Wk, Wv):
    in_maps = _prep_inputs(x, Wq, Wk, Wv)
    res = _get_runner()(in_maps)
    out = np.empty((B, S, D), dtype=np.float32)
    for c in range(N_CORES):
        b, h = divmod(c, 2)
        out[b, h * SQ:(h + 1) * SQ, :] = res["out"][c]
    return out
